# revision 24
# baseline (speedup 1.0000x reference)
"""Causal self-attention with RoPE on 8 TRN2 NeuronCores.

Problem: B=4, T=2048, D=1024, 16 heads x 64 dims, fp32, causal, RoPE.

Sharding: (batch b, head-group g) -> core b*2+g. Each core computes the
full sequence for 8 heads of one batch plus that group's partial output
projection; the host sums the two partial projections per batch
(the "all-reduce" of the tensor-parallel split, done host-side).

Per-core layout strategy (everything keeps the contraction dim on SBUF
partitions, so no on-device transposes are needed):
  - host supplies x^T [D, T] and pre-transposed weights wqT/wkT/wvT [D, 512],
    woT [512, D] in bf16
  - q^T, k^T computed as [512 hdim, T] (pair-tiles of 128 partitions =
    2 heads x 64 dims); v computed in natural [T, 512] layout directly
  - RoPE applied to q^T/k^T in fp32: partition-half swap via SBUF->SBUF DMA,
    then two multiplies + add on DVE with host cos/sin tables
  - scores S^T[j,i] = k^T.T @ q^T per head; the two K=64 heads of a pair run
    concurrently in the PE array via tile_position row groups (0,0)/(64,0)
  - exp on ScalarE (scale=1/8 folded in, bf16 out); causal diagonal blocks
    masked by adding a -1e30 triangle to the first 128 columns pre-exp
  - AV: lhsT = [v_h | ones] [j, 65] -> out [65, i] = attn^T rows 0..63 plus
    the softmax denominator in row 64, accumulated over j tiles in PSUM
  - normalize via 1/x = exp(-ln x) on ScalarE with a DRAM-bounce partition
    broadcast; attn^T written in bf16 (head B of each pair lands on
    partitions 64..127 via a small DMA shift)
  - output projection contracts attn^T pair-tiles against woT k-tiles

Measured on 8 NeuronCores: HW exec ~452 us, max rel err ~3.9e-3 vs the
fp32 JAX reference (bf16 input-rounding noise floor).
"""

import numpy as np
import ml_dtypes

import concourse.bass as bass
import concourse.tile as tile
import concourse.mybir as mybir

F32 = mybir.dt.float32
BF16 = mybir.dt.bfloat16
F32R = mybir.dt.float32r

B, T, D = 4, 2048, 1024
NUM_HEADS, HEAD_DIM = 16, 64
ROPE_THETA = 10000.0

G = 512          # head dims per core (8 heads)
HPC = 8          # heads per core
PAIRS = 4        # pair-tiles (2 heads / 128 partitions)
KT = D // 128    # k-tiles over D
TC = 512         # i-chunk width
NCHUNK = T // TC
JT = T // 128    # j-tiles
TT = T // 128    # t-tiles
N_CORES = 8

MASK_NEG = -1.0e30

# Projection / output matmul input dtype. bf16: full PE rate.
DT_PROJ = BF16
# Scores (q,k) matmul dtype. bf16 enables row-packing two K=64 heads into
# one PE pass via tile_position (f32r packing mis-executes on HW).
DT_SCORE = BF16
# probs & v dtype for the AV matmul.
DT_AV = BF16


def _split_multi_waits(nc, max_waits=1):
    """This walrus build rejects >1 sync-wait per instruction; spill extras
    onto same-engine NoOps placed just before."""
    counter = [0]
    for func in nc.m.functions:
        for bb in func.blocks:
            insts = bb.instructions
            if not any(
                ins.sync_info is not None and len(ins.sync_info.on_wait) > max_waits
                for ins in insts
            ):
                continue
            new_list = []
            for ins in insts:
                si = ins.sync_info
                if si is None or len(si.on_wait) <= max_waits:
                    new_list.append(ins)
                    continue
                waits = list(si.on_wait)
                spill, keep = waits[:-max_waits], waits[-max_waits:]
                for w in spill:
                    counter[0] += 1
                    new_list.append(
                        mybir.InstNoOp(
                            name=f"waitnop-{counter[0]}",
                            engine=ins.engine,
                            ins=[],
                            outs=[],
                            sync_info=mybir.SyncInfo(on_wait=[w], on_update=[]),
                        )
                    )
                ins.sync_info = mybir.SyncInfo(on_wait=keep, on_update=list(si.on_update))
                new_list.append(ins)
            bb.instructions = new_list


def build_kernel():
    nc = bass.Bass()

    xT = nc.dram_tensor("xT", [D, T], DT_PROJ, kind="ExternalInput")
    wqT = nc.dram_tensor("wqT", [D, G], DT_PROJ, kind="ExternalInput")
    wkT = nc.dram_tensor("wkT", [D, G], DT_PROJ, kind="ExternalInput")
    wvT = nc.dram_tensor("wvT", [D, G], DT_PROJ, kind="ExternalInput")
    woT = nc.dram_tensor("woT", [G, D], DT_PROJ, kind="ExternalInput")
    cos2 = nc.dram_tensor("cos2", [128, T], F32, kind="ExternalInput")
    sin2 = nc.dram_tensor("sin2", [128, T], F32, kind="ExternalInput")
    trimask = nc.dram_tensor("trimask", [128, 128], F32, kind="ExternalInput")
    out = nc.dram_tensor("out", [T, D], F32, kind="ExternalOutput")

    with tile.TileContext(nc) as tc:
        with (
            tc.tile_pool(name="const", bufs=1) as cpool,
            tc.tile_pool(name="qk", bufs=1) as qkpool,
            tc.tile_pool(name="vext", bufs=1) as vpool,
            tc.tile_pool(name="attn", bufs=1) as apool,
            tc.tile_pool(name="rope", bufs=2) as rpool,
            tc.tile_pool(name="exps", bufs=6) as epool,
            tc.tile_pool(name="norm", bufs=3) as npool,
            tc.tile_pool(name="outp", bufs=1) as opool,
            tc.tile_pool(name="dramb", bufs=2, space="DRAM") as dpool,
            tc.tile_pool(name="mm", bufs=2, space="PSUM") as mmps,
            tc.tile_pool(name="sp", bufs=4, space="PSUM") as spps,
            tc.tile_pool(name="ap", bufs=2, space="PSUM") as apps,
        ):
            # ---- resident loads ----
            wq_sb = cpool.tile([128, KT, G], DT_PROJ, name="wq_sb")
            nc.sync.dma_start(wq_sb[:], wqT.rearrange("(k p) g -> p k g", p=128))
            wk_sb = cpool.tile([128, KT, G], DT_PROJ, name="wk_sb")
            nc.sync.dma_start(wk_sb[:], wkT.rearrange("(k p) g -> p k g", p=128))
            wv_sb = cpool.tile([128, KT, G], DT_PROJ, name="wv_sb")
            nc.sync.dma_start(wv_sb[:], wvT.rearrange("(k p) g -> p k g", p=128))
            wo_sb = cpool.tile([128, PAIRS, D], DT_PROJ, name="wo_sb")
            nc.sync.dma_start(wo_sb[:], woT.rearrange("(k p) d -> p k d", p=128))
            cos_sb = cpool.tile([128, T], F32, name="cos_sb")
            sin_sb = cpool.tile([128, T], F32, name="sin_sb")
            nc.sync.dma_start(cos_sb[:], cos2[:])
            nc.sync.dma_start(sin_sb[:], sin2[:])
            tri_sb = cpool.tile([128, 128], F32, name="tri_sb")
            nc.sync.dma_start(tri_sb[:], trimask[:])
            sgn_sb = cpool.tile([128, 1], F32, name="sgn_sb")
            nc.vector.memset(sgn_sb[0:32, :], -1.0)
            nc.vector.memset(sgn_sb[32:64, :], 1.0)
            nc.vector.memset(sgn_sb[64:96, :], -1.0)
            nc.vector.memset(sgn_sb[96:128, :], 1.0)

            qrot = qkpool.tile([128, PAIRS, T], DT_SCORE, name="qrot")
            krot = qkpool.tile([128, PAIRS, T], DT_SCORE, name="krot")
            v_ext = vpool.tile([128, TT, HPC, 65], DT_AV, name="v_ext")
            attnT = apool.tile([128, PAIRS, T], DT_PROJ, name="attnT")

            # ---- projections + RoPE, streaming xT per T-chunk ----
            xT_r = xT.rearrange("(k p) t -> p k t", p=128)
            for c in range(NCHUNK):
                csl = bass.ts(c, TC)
                xc = rpool.tile([128, KT, TC], DT_PROJ, name="xc", tag="xc")
                nc.sync.dma_start(xc[:], xT_r[:, :, csl])
                for p in range(PAIRS):
                    for w_sb, rot in ((wq_sb, qrot), (wk_sb, krot)):
                        ps = mmps.tile([128, TC], F32, name="proj_ps", tag="mmps")
                        for k in range(KT):
                            nc.tensor.matmul(
                                ps[:],
                                w_sb[:, k, bass.ts(p, 128)],
                                xc[:, k, :],
                                start=(k == 0),
                                stop=(k == KT - 1),
                            )
                        # rope: rot = ps*cos + swap(ps)*sgn*sin
                        pf = rpool.tile([128, TC], F32, name="pf", tag="pf")
                        nc.vector.tensor_copy(pf[:], ps[:])
                        sw = rpool.tile([128, TC], F32, name="sw", tag="sw")
                        for blk in range(4):
                            src = (blk ^ 1) * 32
                            nc.sync.dma_start(
                                sw[blk * 32 : blk * 32 + 32, :],
                                pf[src : src + 32, :],
                            )
                        t2 = rpool.tile([128, TC], F32, name="t2", tag="t2")
                        nc.vector.tensor_mul(t2[:], ps[:], cos_sb[:, csl])
                        nc.vector.scalar_tensor_tensor(
                            out=sw[:],
                            in0=sw[:],
                            scalar=sgn_sb[:],
                            in1=sin_sb[:, csl],
                            op0=mybir.AluOpType.mult,
                            op1=mybir.AluOpType.mult,
                        )
                        nc.vector.tensor_add(rot[:, p, csl], sw[:], t2[:])
                # V projection for this chunk's 4 t-tiles (natural layout)
                for tt in range(4):
                    t = 4 * c + tt
                    ps = mmps.tile([128, G], F32, name="v_ps", tag="mmps")
                    for k in range(KT):
                        nc.tensor.matmul(
                            ps[:],
                            xc[:, k, bass.ts(tt, 128)],
                            wv_sb[:, k, :],
                            start=(k == 0),
                            stop=(k == KT - 1),
                        )
                    nc.vector.tensor_copy(
                        v_ext[:, t, :, 0:64],
                        ps[:].rearrange("p (h d) -> p h d", h=HPC),
                    )
                    nc.vector.memset(v_ext[:, t, :, 64:65], 1.0)

            # ---- attention per (pair, chunk), both heads row-packed ----
            # The two K=64 score matmuls of a head pair run concurrently in
            # the PE array via tile_position row groups (0,0)/(64,0).
            for p in range(PAIRS):
                for c in range(NCHUNK):
                    csl = bass.ts(c, TC)
                    atts = [
                        apps.tile([65, TC], F32, name=f"att{hh}_ps", tag="apps")
                        for hh in range(2)
                    ]
                    njt = 4 * c + 4
                    for jt in range(njt):
                        m = jt - 4 * c  # >=0 on diagonal j-tiles
                        sls = [
                            spps.tile([128, TC], F32, name=f"s{hh}_ps", tag="spps")
                            for hh in range(2)
                        ]
                        soff = 128 * m if m > 0 else 0
                        for hh in range(2):
                            hsl = slice(64 * hh, 64 * hh + 64)
                            nc.tensor.matmul(
                                sls[hh][:, soff:TC],
                                krot[hsl, p, bass.ts(jt, 128)],
                                qrot[hsl, p, c * TC + soff : (c + 1) * TC],
                                start=True,
                                stop=True,
                                tile_position=(64 * hh, 0),
                            )
                        for hh in range(2):
                            h = 2 * p + hh
                            sps, att = sls[hh], atts[hh]
                            if m >= 0:
                                off = 128 * m
                                fd = TC - off
                                nc.vector.tensor_add(
                                    sps[:, off : off + 128],
                                    sps[:, off : off + 128],
                                    tri_sb[:],
                                )
                                es = epool.tile([128, TC], DT_AV, name="es", tag="es")
                                nc.scalar.activation(
                                    es[:, 0:fd],
                                    sps[:, off : off + fd],
                                    mybir.ActivationFunctionType.Exp,
                                    scale=0.125,
                                )
                                nc.tensor.matmul(
                                    att[:, off : off + fd],
                                    v_ext[:, jt, h, :],
                                    es[:, 0:fd],
                                    start=(jt == 0),
                                    stop=(jt == njt - 1),
                                )
                            else:
                                es = epool.tile([128, TC], DT_AV, name="es", tag="es")
                                nc.scalar.activation(
                                    es[:],
                                    sps[:],
                                    mybir.ActivationFunctionType.Exp,
                                    scale=0.125,
                                )
                                nc.tensor.matmul(
                                    att[:],
                                    v_ext[:, jt, h, :],
                                    es[:],
                                    start=(jt == 0),
                                    stop=(jt == njt - 1),
                                )
                    # normalize both heads: 1/denom = exp(-ln(denom)) on ACT
                    # (custom-DVE reciprocal unsupported; engines lane-locked).
                    # Ln on the psum denom row (lane 64), DRAM bounce
                    # broadcasts to 64 partitions, Exp(scale=-1) gives 1/x.
                    for hh in range(2):
                        att = atts[hh]
                        nrm = npool.tile([128, TC], F32, name="nrm", tag="nrm")
                        nc.scalar.activation(
                            nrm[64:65, :],
                            att[64:65, :],
                            mybir.ActivationFunctionType.Ln,
                        )
                        rbc = npool.tile([64, TC], F32, name="rbc", tag="rbc")
                        dscr = dpool.tile([1, TC], F32, name="dscr", tag="dscr")
                        nc.sync.dma_start(dscr[:], nrm[64:65, :])
                        dsrc = dscr[:]
                        nc.sync.dma_start(
                            rbc[:],
                            bass.AP(
                                tensor=dsrc.tensor,
                                offset=dsrc.offset,
                                ap=[[0, 64]] + dsrc.ap[1:],
                            ),
                        )
                        nc.scalar.activation(
                            rbc[:],
                            rbc[:],
                            mybir.ActivationFunctionType.Exp,
                            scale=-1.0,
                        )
                        if hh == 0:
                            nc.vector.tensor_mul(
                                attnT[0:64, p, csl], att[0:64, :], rbc[:]
                            )
                        else:
                            btmp = npool.tile([64, TC], DT_PROJ, name="btmp", tag="btmp")
                            nc.vector.tensor_mul(btmp[:], att[0:64, :], rbc[:])
                            nc.sync.dma_start(attnT[64:128, p, csl], btmp[:])

            # ---- output projection ----
            for t in range(TT):
                tsl = bass.ts(t, 128)
                ob = opool.tile([128, D], F32, name="ob", tag="ob")
                for dc in range(2):
                    dsl = bass.ts(dc, 512)
                    ps = mmps.tile([128, 512], F32, name="o_ps", tag="mmps")
                    for p in range(PAIRS):
                        nc.tensor.matmul(
                            ps[:],
                            attnT[:, p, tsl],
                            wo_sb[:, p, dsl],
                            start=(p == 0),
                            stop=(p == PAIRS - 1),
                        )
                    nc.vector.tensor_copy(ob[:, dsl], ps[:])
                nc.sync.dma_start(out[t * 128 : t * 128 + 128, :], ob[:])

    _split_multi_waits(nc)
    return nc


def _round_tf32(x):
    u = np.ascontiguousarray(x, dtype=np.float32).view(np.uint32)
    rb = (u >> 13) & 1
    u = (u + 0x0FFF + rb) & np.uint32(0xFFFFE000)
    return u.view(np.float32)


def _to_dt(x, dt):
    if dt == BF16:
        return np.ascontiguousarray(x).astype(ml_dtypes.bfloat16)
    if dt == F32R:
        return _round_tf32(np.ascontiguousarray(x, dtype=np.float32))
    return np.ascontiguousarray(x, dtype=np.float32)


def _rope_tables():
    inv_freq = 1.0 / ROPE_THETA ** (np.arange(0, HEAD_DIM, 2, dtype=np.float64) / HEAD_DIM)
    freqs = np.outer(np.arange(T, dtype=np.float64), inv_freq)  # [T, 32]
    cos_t = np.cos(freqs).T.astype(np.float32)  # [32, T]
    sin_t = np.sin(freqs).T.astype(np.float32)
    cos2 = np.concatenate([cos_t, cos_t, cos_t, cos_t], axis=0)  # [128, T]
    sin2 = np.concatenate([sin_t, sin_t, sin_t, sin_t], axis=0)
    return np.ascontiguousarray(cos2), np.ascontiguousarray(sin2)


def _trimask():
    j = np.arange(128)[:, None]
    c = np.arange(128)[None, :]
    return np.where(j <= c, 0.0, MASK_NEG).astype(np.float32)


_NC_CACHE = {}
LAST_RESULTS = None  # BassKernelResults of the most recent kernel() call


def kernel(x, wq, wk, wv, wo):
    global LAST_RESULTS
    from concourse.bass_utils import run_bass_kernel_spmd

    x = np.asarray(x, dtype=np.float32)
    wq = np.asarray(wq, dtype=np.float32)
    wk = np.asarray(wk, dtype=np.float32)
    wv = np.asarray(wv, dtype=np.float32)
    wo = np.asarray(wo, dtype=np.float32)

    cos2, sin2 = _rope_tables()
    tri = _trimask()

    in_maps = []
    for core in range(N_CORES):
        b, g = core // 2, core % 2
        gs = slice(G * g, G * g + G)
        in_maps.append(
            {
                "xT": _to_dt(x[b].T, DT_PROJ),
                "wqT": _to_dt(wq[gs].T, DT_PROJ),
                "wkT": _to_dt(wk[gs].T, DT_PROJ),
                "wvT": _to_dt(wv[gs].T, DT_PROJ),
                "woT": _to_dt(wo[:, gs].T, DT_PROJ),
                "cos2": cos2,
                "sin2": sin2,
                "trimask": tri,
            }
        )

    if "nc" not in _NC_CACHE:
        _NC_CACHE["nc"] = build_kernel()
    nc = _NC_CACHE["nc"]

    res = run_bass_kernel_spmd(nc, in_maps, core_ids=list(range(N_CORES)))
    LAST_RESULTS = res
    outs = [r["out"] for r in res.results]
    full = np.empty((B, T, D), dtype=np.float32)
    for b in range(B):
        full[b] = (
            outs[2 * b].astype(np.float64) + outs[2 * b + 1].astype(np.float64)
        ).astype(np.float32)
    return full


# revision 25
# speedup vs baseline: 1.0887x; 1.0887x over previous
"""Causal self-attention with RoPE on 8 TRN2 NeuronCores.

Problem: B=4, T=2048, D=1024, 16 heads x 64 dims, fp32, causal, RoPE.

Sharding: (batch b, head-group g) -> core b*2+g. Each core computes the
full sequence for 8 heads of one batch plus that group's partial output
projection; the host sums the two partial projections per batch
(the "all-reduce" of the tensor-parallel split, done host-side).

Per-core layout strategy (everything keeps the contraction dim on SBUF
partitions, so no on-device transposes are needed):
  - host supplies x^T [D, T] and pre-transposed weights wqT/wkT/wvT [D, 512],
    woT [512, D] in bf16
  - q^T, k^T computed as [512 hdim, T] (pair-tiles of 128 partitions =
    2 heads x 64 dims); v computed in natural [T, 512] layout directly
  - RoPE applied to q^T/k^T in fp32: partition-half swap via SBUF->SBUF DMA,
    then two multiplies + add on DVE with host cos/sin tables
  - scores S^T[j,i] = k^T.T @ q^T per head; the two K=64 heads of a pair run
    concurrently in the PE array via tile_position row groups (0,0)/(64,0)
  - exp on ScalarE (scale=1/8 folded in, bf16 out); causal diagonal blocks
    masked by adding a -1e30 triangle to the first 128 columns pre-exp
  - AV: lhsT = [v_h | ones] [j, 65] -> out [65, i] = attn^T rows 0..63 plus
    the softmax denominator in row 64, accumulated over j tiles in PSUM
  - normalize via 1/x = exp(-ln x) on ScalarE with a DRAM-bounce partition
    broadcast; attn^T written in bf16 (head B of each pair lands on
    partitions 64..127 via a small DMA shift)
  - output projection contracts attn^T pair-tiles against woT k-tiles

Measured on 8 NeuronCores: HW exec ~452 us, max rel err ~3.9e-3 vs the
fp32 JAX reference (bf16 input-rounding noise floor).
"""

import numpy as np
import ml_dtypes

import concourse.bass as bass
import concourse.tile as tile
import concourse.mybir as mybir

F32 = mybir.dt.float32
BF16 = mybir.dt.bfloat16
F32R = mybir.dt.float32r

B, T, D = 4, 2048, 1024
NUM_HEADS, HEAD_DIM = 16, 64
ROPE_THETA = 10000.0

G = 512          # head dims per core (8 heads)
HPC = 8          # heads per core
PAIRS = 4        # pair-tiles (2 heads / 128 partitions)
KT = D // 128    # k-tiles over D
TC = 512         # i-chunk width
NCHUNK = T // TC
JT = T // 128    # j-tiles
TT = T // 128    # t-tiles
N_CORES = 8

MASK_NEG = -1.0e30

# Projection / output matmul input dtype. bf16: full PE rate.
DT_PROJ = BF16
# Scores (q,k) matmul dtype. bf16 enables row-packing two K=64 heads into
# one PE pass via tile_position (f32r packing mis-executes on HW).
DT_SCORE = BF16
# probs & v dtype for the AV matmul.
DT_AV = BF16


def _split_multi_waits(nc, max_waits=1):
    """This walrus build rejects >1 sync-wait per instruction; spill extras
    onto same-engine NoOps placed just before."""
    counter = [0]
    for func in nc.m.functions:
        for bb in func.blocks:
            insts = bb.instructions
            if not any(
                ins.sync_info is not None and len(ins.sync_info.on_wait) > max_waits
                for ins in insts
            ):
                continue
            new_list = []
            for ins in insts:
                si = ins.sync_info
                if si is None or len(si.on_wait) <= max_waits:
                    new_list.append(ins)
                    continue
                waits = list(si.on_wait)
                spill, keep = waits[:-max_waits], waits[-max_waits:]
                for w in spill:
                    counter[0] += 1
                    new_list.append(
                        mybir.InstNoOp(
                            name=f"waitnop-{counter[0]}",
                            engine=ins.engine,
                            ins=[],
                            outs=[],
                            sync_info=mybir.SyncInfo(on_wait=[w], on_update=[]),
                        )
                    )
                ins.sync_info = mybir.SyncInfo(on_wait=keep, on_update=list(si.on_update))
                new_list.append(ins)
            bb.instructions = new_list


def build_kernel():
    nc = bass.Bass()

    xT = nc.dram_tensor("xT", [D, T], DT_PROJ, kind="ExternalInput")
    wqT = nc.dram_tensor("wqT", [D, G], DT_PROJ, kind="ExternalInput")
    wkT = nc.dram_tensor("wkT", [D, G], DT_PROJ, kind="ExternalInput")
    wvT = nc.dram_tensor("wvT", [D, G], DT_PROJ, kind="ExternalInput")
    woT = nc.dram_tensor("woT", [G, D], DT_PROJ, kind="ExternalInput")
    cos2 = nc.dram_tensor("cos2", [128, T], F32, kind="ExternalInput")
    sin2 = nc.dram_tensor("sin2", [128, T], F32, kind="ExternalInput")
    trimask = nc.dram_tensor("trimask", [128, 128], F32, kind="ExternalInput")
    out = nc.dram_tensor("out", [T, D], F32, kind="ExternalOutput")

    with tile.TileContext(nc) as tc:
        with (
            tc.tile_pool(name="const", bufs=1) as cpool,
            tc.tile_pool(name="qk", bufs=1) as qkpool,
            tc.tile_pool(name="vext", bufs=1) as vpool,
            tc.tile_pool(name="attn", bufs=1) as apool,
            tc.tile_pool(name="rope", bufs=2) as rpool,
            tc.tile_pool(name="exps", bufs=6) as epool,
            tc.tile_pool(name="norm", bufs=3) as npool,
            tc.tile_pool(name="outp", bufs=1) as opool,
            tc.tile_pool(name="dramb", bufs=2, space="DRAM") as dpool,
            tc.tile_pool(name="mm", bufs=2, space="PSUM") as mmps,
            tc.tile_pool(name="sp", bufs=3, space="PSUM") as spps,
            tc.tile_pool(name="ap", bufs=3, space="PSUM") as apps,
        ):
            # ---- resident loads ----
            wq_sb = cpool.tile([128, KT, G], DT_PROJ, name="wq_sb")
            nc.sync.dma_start(wq_sb[:], wqT.rearrange("(k p) g -> p k g", p=128))
            wk_sb = cpool.tile([128, KT, G], DT_PROJ, name="wk_sb")
            nc.sync.dma_start(wk_sb[:], wkT.rearrange("(k p) g -> p k g", p=128))
            wv_sb = cpool.tile([128, KT, G], DT_PROJ, name="wv_sb")
            nc.sync.dma_start(wv_sb[:], wvT.rearrange("(k p) g -> p k g", p=128))
            wo_sb = cpool.tile([128, PAIRS, D], DT_PROJ, name="wo_sb")
            nc.sync.dma_start(wo_sb[:], woT.rearrange("(k p) d -> p k d", p=128))
            cos_sb = cpool.tile([128, T], F32, name="cos_sb")
            sin_sb = cpool.tile([128, T], F32, name="sin_sb")
            nc.sync.dma_start(cos_sb[:], cos2[:])
            nc.sync.dma_start(sin_sb[:], sin2[:])
            tri_sb = cpool.tile([128, 128], F32, name="tri_sb")
            nc.sync.dma_start(tri_sb[:], trimask[:])
            sgn_sb = cpool.tile([128, 1], F32, name="sgn_sb")
            nc.vector.memset(sgn_sb[0:32, :], -1.0)
            nc.vector.memset(sgn_sb[32:64, :], 1.0)
            nc.vector.memset(sgn_sb[64:96, :], -1.0)
            nc.vector.memset(sgn_sb[96:128, :], 1.0)

            qrot = qkpool.tile([128, PAIRS, T], DT_SCORE, name="qrot")
            krot = qkpool.tile([128, PAIRS, T], DT_SCORE, name="krot")
            v_ext = vpool.tile([128, TT, HPC, 65], DT_AV, name="v_ext")
            attnT = apool.tile([128, PAIRS, T], DT_PROJ, name="attnT")

            # ---- projections + RoPE, streaming xT per T-chunk ----
            xT_r = xT.rearrange("(k p) t -> p k t", p=128)
            for c in range(NCHUNK):
                csl = bass.ts(c, TC)
                xc = rpool.tile([128, KT, TC], DT_PROJ, name="xc", tag="xc")
                nc.sync.dma_start(xc[:], xT_r[:, :, csl])
                for p in range(PAIRS):
                    for w_sb, rot in ((wq_sb, qrot), (wk_sb, krot)):
                        ps = mmps.tile([128, TC], F32, name="proj_ps", tag="mmps")
                        for k in range(KT):
                            nc.tensor.matmul(
                                ps[:],
                                w_sb[:, k, bass.ts(p, 128)],
                                xc[:, k, :],
                                start=(k == 0),
                                stop=(k == KT - 1),
                            )
                        # rope: rot = ps*cos + swap(ps)*sgn*sin
                        pf = rpool.tile([128, TC], F32, name="pf", tag="pf")
                        nc.vector.tensor_copy(pf[:], ps[:])
                        sw = rpool.tile([128, TC], F32, name="sw", tag="sw")
                        for blk in range(4):
                            src = (blk ^ 1) * 32
                            nc.sync.dma_start(
                                sw[blk * 32 : blk * 32 + 32, :],
                                pf[src : src + 32, :],
                            )
                        t2 = rpool.tile([128, TC], F32, name="t2", tag="t2")
                        nc.vector.tensor_mul(t2[:], ps[:], cos_sb[:, csl])
                        nc.vector.scalar_tensor_tensor(
                            out=sw[:],
                            in0=sw[:],
                            scalar=sgn_sb[:],
                            in1=sin_sb[:, csl],
                            op0=mybir.AluOpType.mult,
                            op1=mybir.AluOpType.mult,
                        )
                        nc.vector.tensor_add(rot[:, p, csl], sw[:], t2[:])
                # V projection for this chunk's 4 t-tiles (natural layout)
                for tt in range(4):
                    t = 4 * c + tt
                    ps = mmps.tile([128, G], F32, name="v_ps", tag="mmps")
                    for k in range(KT):
                        nc.tensor.matmul(
                            ps[:],
                            xc[:, k, bass.ts(tt, 128)],
                            wv_sb[:, k, :],
                            start=(k == 0),
                            stop=(k == KT - 1),
                        )
                    nc.vector.tensor_copy(
                        v_ext[:, t, :, 0:64],
                        ps[:].rearrange("p (h d) -> p h d", h=HPC),
                    )
                    nc.vector.memset(v_ext[:, t, :, 64:65], 1.0)

            # ---- attention per (pair, chunk), both heads row-packed ----
            # The two K=64 score matmuls of a head pair run concurrently in
            # the PE array via tile_position row groups (0,0)/(64,0).
            for p in range(PAIRS):
                for c in range(NCHUNK):
                    csl = bass.ts(c, TC)
                    atts = [
                        apps.tile([65, TC], F32, name=f"att{hh}_ps", tag="apps")
                        for hh in range(2)
                    ]
                    njt = 4 * c + 4
                    for jt in range(njt):
                        m = jt - 4 * c  # >=0 on diagonal j-tiles
                        sls = [
                            spps.tile([128, TC], F32, name=f"s{hh}_ps", tag="spps")
                            for hh in range(2)
                        ]
                        soff = 128 * m if m > 0 else 0
                        for hh in range(2):
                            hsl = slice(64 * hh, 64 * hh + 64)
                            nc.tensor.matmul(
                                sls[hh][:, soff:TC],
                                krot[hsl, p, bass.ts(jt, 128)],
                                qrot[hsl, p, c * TC + soff : (c + 1) * TC],
                                start=True,
                                stop=True,
                                tile_position=(64 * hh, 0),
                            )
                        for hh in range(2):
                            h = 2 * p + hh
                            sps, att = sls[hh], atts[hh]
                            if m >= 0:
                                off = 128 * m
                                fd = TC - off
                                nc.vector.tensor_add(
                                    sps[:, off : off + 128],
                                    sps[:, off : off + 128],
                                    tri_sb[:],
                                )
                                es = epool.tile([128, TC], DT_AV, name="es", tag="es")
                                nc.scalar.activation(
                                    es[:, 0:fd],
                                    sps[:, off : off + fd],
                                    mybir.ActivationFunctionType.Exp,
                                    scale=0.125,
                                )
                                nc.tensor.matmul(
                                    att[:, off : off + fd],
                                    v_ext[:, jt, h, :],
                                    es[:, 0:fd],
                                    start=(jt == 0),
                                    stop=(jt == njt - 1),
                                )
                            else:
                                es = epool.tile([128, TC], DT_AV, name="es", tag="es")
                                nc.scalar.activation(
                                    es[:],
                                    sps[:],
                                    mybir.ActivationFunctionType.Exp,
                                    scale=0.125,
                                )
                                nc.tensor.matmul(
                                    att[:],
                                    v_ext[:, jt, h, :],
                                    es[:],
                                    start=(jt == 0),
                                    stop=(jt == njt - 1),
                                )
                    # normalize both heads: 1/denom = exp(-ln(denom)) on ACT
                    # (custom-DVE reciprocal unsupported; engines lane-locked).
                    # Ln on the psum denom row (lane 64), DRAM bounce
                    # broadcasts to 64 partitions, Exp(scale=-1) gives 1/x.
                    for hh in range(2):
                        att = atts[hh]
                        nrm = npool.tile([128, TC], F32, name="nrm", tag="nrm")
                        nc.scalar.activation(
                            nrm[64:65, :],
                            att[64:65, :],
                            mybir.ActivationFunctionType.Ln,
                        )
                        rbc = npool.tile([64, TC], F32, name="rbc", tag="rbc")
                        dscr = dpool.tile([1, TC], F32, name="dscr", tag="dscr")
                        nc.sync.dma_start(dscr[:], nrm[64:65, :])
                        dsrc = dscr[:]
                        nc.sync.dma_start(
                            rbc[:],
                            bass.AP(
                                tensor=dsrc.tensor,
                                offset=dsrc.offset,
                                ap=[[0, 64]] + dsrc.ap[1:],
                            ),
                        )
                        nc.scalar.activation(
                            rbc[:],
                            rbc[:],
                            mybir.ActivationFunctionType.Exp,
                            scale=-1.0,
                        )
                        if hh == 0:
                            nc.vector.tensor_mul(
                                attnT[0:64, p, csl], att[0:64, :], rbc[:]
                            )
                        else:
                            btmp = npool.tile([64, TC], DT_PROJ, name="btmp", tag="btmp")
                            nc.vector.tensor_mul(btmp[:], att[0:64, :], rbc[:])
                            nc.sync.dma_start(attnT[64:128, p, csl], btmp[:])

            # ---- output projection ----
            for t in range(TT):
                tsl = bass.ts(t, 128)
                ob = opool.tile([128, D], F32, name="ob", tag="ob")
                for dc in range(2):
                    dsl = bass.ts(dc, 512)
                    ps = mmps.tile([128, 512], F32, name="o_ps", tag="mmps")
                    for p in range(PAIRS):
                        nc.tensor.matmul(
                            ps[:],
                            attnT[:, p, tsl],
                            wo_sb[:, p, dsl],
                            start=(p == 0),
                            stop=(p == PAIRS - 1),
                        )
                    nc.vector.tensor_copy(ob[:, dsl], ps[:])
                nc.sync.dma_start(out[t * 128 : t * 128 + 128, :], ob[:])

    _split_multi_waits(nc)
    return nc


def _round_tf32(x):
    u = np.ascontiguousarray(x, dtype=np.float32).view(np.uint32)
    rb = (u >> 13) & 1
    u = (u + 0x0FFF + rb) & np.uint32(0xFFFFE000)
    return u.view(np.float32)


def _to_dt(x, dt):
    if dt == BF16:
        return np.ascontiguousarray(x).astype(ml_dtypes.bfloat16)
    if dt == F32R:
        return _round_tf32(np.ascontiguousarray(x, dtype=np.float32))
    return np.ascontiguousarray(x, dtype=np.float32)


def _rope_tables():
    inv_freq = 1.0 / ROPE_THETA ** (np.arange(0, HEAD_DIM, 2, dtype=np.float64) / HEAD_DIM)
    freqs = np.outer(np.arange(T, dtype=np.float64), inv_freq)  # [T, 32]
    cos_t = np.cos(freqs).T.astype(np.float32)  # [32, T]
    sin_t = np.sin(freqs).T.astype(np.float32)
    cos2 = np.concatenate([cos_t, cos_t, cos_t, cos_t], axis=0)  # [128, T]
    sin2 = np.concatenate([sin_t, sin_t, sin_t, sin_t], axis=0)
    return np.ascontiguousarray(cos2), np.ascontiguousarray(sin2)


def _trimask():
    j = np.arange(128)[:, None]
    c = np.arange(128)[None, :]
    return np.where(j <= c, 0.0, MASK_NEG).astype(np.float32)


_NC_CACHE = {}
LAST_RESULTS = None  # BassKernelResults of the most recent kernel() call


def kernel(x, wq, wk, wv, wo):
    global LAST_RESULTS
    from concourse.bass_utils import run_bass_kernel_spmd

    x = np.asarray(x, dtype=np.float32)
    wq = np.asarray(wq, dtype=np.float32)
    wk = np.asarray(wk, dtype=np.float32)
    wv = np.asarray(wv, dtype=np.float32)
    wo = np.asarray(wo, dtype=np.float32)

    cos2, sin2 = _rope_tables()
    tri = _trimask()

    in_maps = []
    for core in range(N_CORES):
        b, g = core // 2, core % 2
        gs = slice(G * g, G * g + G)
        in_maps.append(
            {
                "xT": _to_dt(x[b].T, DT_PROJ),
                "wqT": _to_dt(wq[gs].T, DT_PROJ),
                "wkT": _to_dt(wk[gs].T, DT_PROJ),
                "wvT": _to_dt(wv[gs].T, DT_PROJ),
                "woT": _to_dt(wo[:, gs].T, DT_PROJ),
                "cos2": cos2,
                "sin2": sin2,
                "trimask": tri,
            }
        )

    if "nc" not in _NC_CACHE:
        _NC_CACHE["nc"] = build_kernel()
    nc = _NC_CACHE["nc"]

    res = run_bass_kernel_spmd(nc, in_maps, core_ids=list(range(N_CORES)))
    LAST_RESULTS = res
    outs = [r["out"] for r in res.results]
    full = np.empty((B, T, D), dtype=np.float32)
    for b in range(B):
        full[b] = (
            outs[2 * b].astype(np.float64) + outs[2 * b + 1].astype(np.float64)
        ).astype(np.float32)
    return full


# revision 27
# speedup vs baseline: 1.1971x; 1.0996x over previous
"""Causal self-attention with RoPE on 8 TRN2 NeuronCores.

Problem: B=4, T=2048, D=1024, 16 heads x 64 dims, fp32, causal, RoPE.

Sharding: (batch b, head-group g) -> core b*2+g. Each core computes the
full sequence for 8 heads of one batch plus that group's partial output
projection; the host sums the two partial projections per batch
(the "all-reduce" of the tensor-parallel split, done host-side).

Per-core layout strategy (everything keeps the contraction dim on SBUF
partitions, so no on-device transposes are needed):
  - host supplies x^T [D, T] and pre-transposed weights wqT/wkT/wvT [D, 512],
    woT [512, D] in bf16
  - q^T, k^T computed as [512 hdim, T] (pair-tiles of 128 partitions =
    2 heads x 64 dims); v computed in natural [T, 512] layout directly
  - RoPE applied to q^T/k^T in fp32: partition-half swap via SBUF->SBUF DMA,
    then two multiplies + add on DVE with host cos/sin tables
  - scores S^T[j,i] = k^T.T @ q^T per head; the two K=64 heads of a pair run
    concurrently in the PE array via tile_position row groups (0,0)/(64,0)
  - exp on ScalarE (scale=1/8 folded in, bf16 out); causal diagonal blocks
    masked by adding a -1e30 triangle to the first 128 columns pre-exp
  - AV: lhsT = [v_h | ones] [j, 65] -> out [65, i] = attn^T rows 0..63 plus
    the softmax denominator in row 64, accumulated over j tiles in PSUM
  - normalize via 1/x = exp(-ln x) on ScalarE with a DRAM-bounce partition
    broadcast; attn^T written in bf16 (head B of each pair lands on
    partitions 64..127 via a small DMA shift)
  - output projection contracts attn^T pair-tiles against woT k-tiles

Measured on 8 NeuronCores: HW exec ~455 us, max rel err ~3.9e-3 vs the
fp32 JAX reference (bf16 input-rounding noise floor).
"""

import numpy as np
import ml_dtypes

import concourse.bass as bass
import concourse.tile as tile
import concourse.mybir as mybir

F32 = mybir.dt.float32
BF16 = mybir.dt.bfloat16
F32R = mybir.dt.float32r

B, T, D = 4, 2048, 1024
NUM_HEADS, HEAD_DIM = 16, 64
ROPE_THETA = 10000.0

G = 512          # head dims per core (8 heads)
HPC = 8          # heads per core
PAIRS = 4        # pair-tiles (2 heads / 128 partitions)
KT = D // 128    # k-tiles over D
TC = 512         # i-chunk width
NCHUNK = T // TC
JT = T // 128    # j-tiles
TT = T // 128    # t-tiles
N_CORES = 8

MASK_NEG = -1.0e30

# Projection / output matmul input dtype. bf16: full PE rate.
DT_PROJ = BF16
# Scores (q,k) matmul dtype. bf16 enables row-packing two K=64 heads into
# one PE pass via tile_position (f32r packing mis-executes on HW).
DT_SCORE = BF16
# probs & v dtype for the AV matmul.
DT_AV = BF16


def _split_multi_waits(nc, max_waits=1):
    """This walrus build rejects >1 sync-wait per instruction; spill extras
    onto same-engine NoOps placed just before."""
    counter = [0]
    for func in nc.m.functions:
        for bb in func.blocks:
            insts = bb.instructions
            if not any(
                ins.sync_info is not None and len(ins.sync_info.on_wait) > max_waits
                for ins in insts
            ):
                continue
            new_list = []
            for ins in insts:
                si = ins.sync_info
                if si is None or len(si.on_wait) <= max_waits:
                    new_list.append(ins)
                    continue
                waits = list(si.on_wait)
                spill, keep = waits[:-max_waits], waits[-max_waits:]
                for w in spill:
                    counter[0] += 1
                    new_list.append(
                        mybir.InstNoOp(
                            name=f"waitnop-{counter[0]}",
                            engine=ins.engine,
                            ins=[],
                            outs=[],
                            sync_info=mybir.SyncInfo(on_wait=[w], on_update=[]),
                        )
                    )
                ins.sync_info = mybir.SyncInfo(on_wait=keep, on_update=list(si.on_update))
                new_list.append(ins)
            bb.instructions = new_list


def build_kernel():
    nc = bass.Bass()

    xT = nc.dram_tensor("xT", [D, T], DT_PROJ, kind="ExternalInput")
    wqT = nc.dram_tensor("wqT", [D, G], DT_PROJ, kind="ExternalInput")
    wkT = nc.dram_tensor("wkT", [D, G], DT_PROJ, kind="ExternalInput")
    wvT = nc.dram_tensor("wvT", [D, G], DT_PROJ, kind="ExternalInput")
    woT = nc.dram_tensor("woT", [G, D], DT_PROJ, kind="ExternalInput")
    cos2 = nc.dram_tensor("cos2", [128, T], F32, kind="ExternalInput")
    sin2 = nc.dram_tensor("sin2", [128, T], F32, kind="ExternalInput")
    trimask = nc.dram_tensor("trimask", [128, 128], F32, kind="ExternalInput")
    out = nc.dram_tensor("out", [T, D], F32, kind="ExternalOutput")

    with tile.TileContext(nc) as tc:
        with (
            tc.tile_pool(name="const", bufs=1) as cpool,
            tc.tile_pool(name="qk", bufs=1) as qkpool,
            tc.tile_pool(name="vext", bufs=1) as vpool,
            tc.tile_pool(name="attn", bufs=1) as apool,
            tc.tile_pool(name="rope", bufs=2) as rpool,
            tc.tile_pool(name="exps", bufs=4) as epool,
            tc.tile_pool(name="norm", bufs=3) as npool,
            tc.tile_pool(name="outp", bufs=1) as opool,
            tc.tile_pool(name="dramb", bufs=2, space="DRAM") as dpool,
            tc.tile_pool(name="mm", bufs=2, space="PSUM") as mmps,
            tc.tile_pool(name="sp", bufs=2, space="PSUM") as spps,
            tc.tile_pool(name="ap", bufs=2, space="PSUM") as apps,
        ):
            # ---- resident loads ----
            wq_sb = cpool.tile([128, KT, G], DT_PROJ, name="wq_sb")
            nc.sync.dma_start(wq_sb[:], wqT.rearrange("(k p) g -> p k g", p=128))
            wk_sb = cpool.tile([128, KT, G], DT_PROJ, name="wk_sb")
            nc.sync.dma_start(wk_sb[:], wkT.rearrange("(k p) g -> p k g", p=128))
            wv_sb = cpool.tile([128, KT, G], DT_PROJ, name="wv_sb")
            nc.sync.dma_start(wv_sb[:], wvT.rearrange("(k p) g -> p k g", p=128))
            wo_sb = cpool.tile([128, PAIRS, D], DT_PROJ, name="wo_sb")
            nc.sync.dma_start(wo_sb[:], woT.rearrange("(k p) d -> p k d", p=128))
            cos_sb = cpool.tile([128, T], F32, name="cos_sb")
            sin_sb = cpool.tile([128, T], F32, name="sin_sb")
            nc.sync.dma_start(cos_sb[:], cos2[:])
            nc.sync.dma_start(sin_sb[:], sin2[:])
            tri_sb = cpool.tile([128, 128], F32, name="tri_sb")
            nc.sync.dma_start(tri_sb[:], trimask[:])
            sgn_sb = cpool.tile([128, 1], F32, name="sgn_sb")
            nc.vector.memset(sgn_sb[0:32, :], -1.0)
            nc.vector.memset(sgn_sb[32:64, :], 1.0)
            nc.vector.memset(sgn_sb[64:96, :], -1.0)
            nc.vector.memset(sgn_sb[96:128, :], 1.0)

            qrot = qkpool.tile([128, PAIRS, T], DT_SCORE, name="qrot")
            krot = qkpool.tile([128, PAIRS, T], DT_SCORE, name="krot")
            v_ext = vpool.tile([128, TT, HPC, 65], DT_AV, name="v_ext")
            attnT = apool.tile([128, PAIRS, T], DT_PROJ, name="attnT")

            # ---- projections + RoPE, streaming xT per T-chunk ----
            xT_r = xT.rearrange("(k p) t -> p k t", p=128)
            for c in range(NCHUNK):
                csl = bass.ts(c, TC)
                xc = rpool.tile([128, KT, TC], DT_PROJ, name="xc", tag="xc")
                nc.sync.dma_start(xc[:], xT_r[:, :, csl])
                for p in range(PAIRS):
                    for w_sb, rot in ((wq_sb, qrot), (wk_sb, krot)):
                        ps = mmps.tile([128, TC], F32, name="proj_ps", tag="mmps")
                        for k in range(KT):
                            nc.tensor.matmul(
                                ps[:],
                                w_sb[:, k, bass.ts(p, 128)],
                                xc[:, k, :],
                                start=(k == 0),
                                stop=(k == KT - 1),
                            )
                        # rope: rot = ps*cos + swap(ps)*sgn*sin
                        pf = rpool.tile([128, TC], F32, name="pf", tag="pf")
                        nc.vector.tensor_copy(pf[:], ps[:])
                        sw = rpool.tile([128, TC], F32, name="sw", tag="sw")
                        for blk in range(4):
                            src = (blk ^ 1) * 32
                            nc.sync.dma_start(
                                sw[blk * 32 : blk * 32 + 32, :],
                                pf[src : src + 32, :],
                            )
                        t2 = rpool.tile([128, TC], F32, name="t2", tag="t2")
                        nc.vector.tensor_mul(t2[:], ps[:], cos_sb[:, csl])
                        nc.vector.scalar_tensor_tensor(
                            out=sw[:],
                            in0=sw[:],
                            scalar=sgn_sb[:],
                            in1=sin_sb[:, csl],
                            op0=mybir.AluOpType.mult,
                            op1=mybir.AluOpType.mult,
                        )
                        nc.vector.tensor_add(rot[:, p, csl], sw[:], t2[:])
                # V projection for this chunk's 4 t-tiles (natural layout)
                for tt in range(4):
                    t = 4 * c + tt
                    ps = mmps.tile([128, G], F32, name="v_ps", tag="mmps")
                    for k in range(KT):
                        nc.tensor.matmul(
                            ps[:],
                            xc[:, k, bass.ts(tt, 128)],
                            wv_sb[:, k, :],
                            start=(k == 0),
                            stop=(k == KT - 1),
                        )
                    nc.vector.tensor_copy(
                        v_ext[:, t, :, 0:64],
                        ps[:].rearrange("p (h d) -> p h d", h=HPC),
                    )
                    nc.vector.memset(v_ext[:, t, :, 64:65], 1.0)

            # ---- attention per (pair, chunk), both heads row-packed ----
            # The two K=64 score matmuls of a head pair run concurrently in
            # the PE array via tile_position row groups (0,0)/(64,0).
            for p in range(PAIRS):
                for c in range(NCHUNK):
                    csl = bass.ts(c, TC)
                    atts = [
                        apps.tile([65, TC], F32, name=f"att{hh}_ps", tag="apps")
                        for hh in range(2)
                    ]
                    njt = 4 * c + 4
                    for jt in range(njt):
                        m = jt - 4 * c  # >=0 on diagonal j-tiles
                        # both heads' scores in one 2-bank psum tile so a
                        # single FD-doubled exp covers the pair
                        sAB = spps.tile([128, 2, TC], F32, name="s_ps", tag="spps")
                        soff = 128 * m if m > 0 else 0
                        for hh in range(2):
                            hsl = slice(64 * hh, 64 * hh + 64)
                            nc.tensor.matmul(
                                sAB[:, hh, soff:TC],
                                krot[hsl, p, bass.ts(jt, 128)],
                                qrot[hsl, p, c * TC + soff : (c + 1) * TC],
                                start=True,
                                stop=True,
                                tile_position=(64 * hh, 0),
                            )
                        off = 128 * m
                        fd = TC - off
                        if m >= 0:
                            for hh in range(2):
                                nc.vector.tensor_add(
                                    sAB[:, hh, off : off + 128],
                                    sAB[:, hh, off : off + 128],
                                    tri_sb[:],
                                )
                            es = epool.tile([128, 2, TC], DT_AV, name="es", tag="es")
                            nc.scalar.activation(
                                es[:, :, 0:fd],
                                sAB[:, :, off : off + fd],
                                mybir.ActivationFunctionType.Exp,
                                scale=0.125,
                            )
                            for hh in range(2):
                                nc.tensor.matmul(
                                    atts[hh][:, off : off + fd],
                                    v_ext[:, jt, 2 * p + hh, :],
                                    es[:, hh, 0:fd],
                                    start=(jt == 0),
                                    stop=(jt == njt - 1),
                                )
                        else:
                            es = epool.tile([128, 2, TC], DT_AV, name="es", tag="es")
                            nc.scalar.activation(
                                es[:],
                                sAB[:],
                                mybir.ActivationFunctionType.Exp,
                                scale=0.125,
                            )
                            for hh in range(2):
                                nc.tensor.matmul(
                                    atts[hh][:],
                                    v_ext[:, jt, 2 * p + hh, :],
                                    es[:, hh, :],
                                    start=(jt == 0),
                                    stop=(jt == njt - 1),
                                )
                    # normalize both heads: 1/denom = exp(-ln(denom)) on ACT
                    # (custom-DVE reciprocal unsupported; engines lane-locked).
                    # Ln on the psum denom row (lane 64), DRAM bounce
                    # broadcasts to 64 partitions, Exp(scale=-1) gives 1/x.
                    for hh in range(2):
                        att = atts[hh]
                        nrm = npool.tile([128, TC], F32, name="nrm", tag="nrm")
                        nc.scalar.activation(
                            nrm[64:65, :],
                            att[64:65, :],
                            mybir.ActivationFunctionType.Ln,
                        )
                        rbc = npool.tile([64, TC], F32, name="rbc", tag="rbc")
                        dscr = dpool.tile([1, TC], F32, name="dscr", tag="dscr")
                        nc.sync.dma_start(dscr[:], nrm[64:65, :])
                        dsrc = dscr[:]
                        nc.sync.dma_start(
                            rbc[:],
                            bass.AP(
                                tensor=dsrc.tensor,
                                offset=dsrc.offset,
                                ap=[[0, 64]] + dsrc.ap[1:],
                            ),
                        )
                        nc.scalar.activation(
                            rbc[:],
                            rbc[:],
                            mybir.ActivationFunctionType.Exp,
                            scale=-1.0,
                        )
                        if hh == 0:
                            nc.vector.tensor_mul(
                                attnT[0:64, p, csl], att[0:64, :], rbc[:]
                            )
                        else:
                            btmp = npool.tile([64, TC], DT_PROJ, name="btmp", tag="btmp")
                            nc.vector.tensor_mul(btmp[:], att[0:64, :], rbc[:])
                            nc.sync.dma_start(attnT[64:128, p, csl], btmp[:])

            # ---- output projection ----
            for t in range(TT):
                tsl = bass.ts(t, 128)
                ob = opool.tile([128, D], F32, name="ob", tag="ob")
                for dc in range(2):
                    dsl = bass.ts(dc, 512)
                    ps = mmps.tile([128, 512], F32, name="o_ps", tag="mmps")
                    for p in range(PAIRS):
                        nc.tensor.matmul(
                            ps[:],
                            attnT[:, p, tsl],
                            wo_sb[:, p, dsl],
                            start=(p == 0),
                            stop=(p == PAIRS - 1),
                        )
                    nc.vector.tensor_copy(ob[:, dsl], ps[:])
                nc.sync.dma_start(out[t * 128 : t * 128 + 128, :], ob[:])

    _split_multi_waits(nc)
    return nc


def _round_tf32(x):
    u = np.ascontiguousarray(x, dtype=np.float32).view(np.uint32)
    rb = (u >> 13) & 1
    u = (u + 0x0FFF + rb) & np.uint32(0xFFFFE000)
    return u.view(np.float32)


def _to_dt(x, dt):
    if dt == BF16:
        return np.ascontiguousarray(x).astype(ml_dtypes.bfloat16)
    if dt == F32R:
        return _round_tf32(np.ascontiguousarray(x, dtype=np.float32))
    return np.ascontiguousarray(x, dtype=np.float32)


def _rope_tables():
    inv_freq = 1.0 / ROPE_THETA ** (np.arange(0, HEAD_DIM, 2, dtype=np.float64) / HEAD_DIM)
    freqs = np.outer(np.arange(T, dtype=np.float64), inv_freq)  # [T, 32]
    cos_t = np.cos(freqs).T.astype(np.float32)  # [32, T]
    sin_t = np.sin(freqs).T.astype(np.float32)
    cos2 = np.concatenate([cos_t, cos_t, cos_t, cos_t], axis=0)  # [128, T]
    sin2 = np.concatenate([sin_t, sin_t, sin_t, sin_t], axis=0)
    return np.ascontiguousarray(cos2), np.ascontiguousarray(sin2)


def _trimask():
    j = np.arange(128)[:, None]
    c = np.arange(128)[None, :]
    return np.where(j <= c, 0.0, MASK_NEG).astype(np.float32)


_NC_CACHE = {}
LAST_RESULTS = None  # BassKernelResults of the most recent kernel() call


def kernel(x, wq, wk, wv, wo):
    global LAST_RESULTS
    from concourse.bass_utils import run_bass_kernel_spmd

    x = np.asarray(x, dtype=np.float32)
    wq = np.asarray(wq, dtype=np.float32)
    wk = np.asarray(wk, dtype=np.float32)
    wv = np.asarray(wv, dtype=np.float32)
    wo = np.asarray(wo, dtype=np.float32)

    cos2, sin2 = _rope_tables()
    tri = _trimask()

    in_maps = []
    for core in range(N_CORES):
        b, g = core // 2, core % 2
        gs = slice(G * g, G * g + G)
        in_maps.append(
            {
                "xT": _to_dt(x[b].T, DT_PROJ),
                "wqT": _to_dt(wq[gs].T, DT_PROJ),
                "wkT": _to_dt(wk[gs].T, DT_PROJ),
                "wvT": _to_dt(wv[gs].T, DT_PROJ),
                "woT": _to_dt(wo[:, gs].T, DT_PROJ),
                "cos2": cos2,
                "sin2": sin2,
                "trimask": tri,
            }
        )

    if "nc" not in _NC_CACHE:
        _NC_CACHE["nc"] = build_kernel()
    nc = _NC_CACHE["nc"]

    res = run_bass_kernel_spmd(nc, in_maps, core_ids=list(range(N_CORES)))
    LAST_RESULTS = res
    outs = [r["out"] for r in res.results]
    full = np.empty((B, T, D), dtype=np.float32)
    for b in range(B):
        full[b] = (
            outs[2 * b].astype(np.float64) + outs[2 * b + 1].astype(np.float64)
        ).astype(np.float32)
    return full


# revision 28
# speedup vs baseline: 1.2042x; 1.0060x over previous
"""Causal self-attention with RoPE on 8 TRN2 NeuronCores.

Problem: B=4, T=2048, D=1024, 16 heads x 64 dims, fp32, causal, RoPE.

Sharding: (batch b, head-group g) -> core b*2+g. Each core computes the
full sequence for 8 heads of one batch plus that group's partial output
projection; the host sums the two partial projections per batch
(the "all-reduce" of the tensor-parallel split, done host-side).

Per-core layout strategy (everything keeps the contraction dim on SBUF
partitions, so no on-device transposes are needed):
  - host supplies x^T [D, T] and pre-transposed weights wqT/wkT/wvT [D, 512],
    woT [512, D] in bf16
  - q^T, k^T computed as [512 hdim, T] (pair-tiles of 128 partitions =
    2 heads x 64 dims); v computed in natural [T, 512] layout directly
  - RoPE applied to q^T/k^T in fp32: partition-half swap via SBUF->SBUF DMA,
    then two multiplies + add on DVE with host cos/sin tables
  - scores S^T[j,i] = k^T.T @ q^T per head; the two K=64 heads of a pair run
    concurrently in the PE array via tile_position row groups (0,0)/(64,0)
  - exp on ScalarE (scale=1/8 folded in, bf16 out); causal diagonal blocks
    masked by adding a -1e30 triangle to the first 128 columns pre-exp
  - AV: lhsT = [v_h | ones] [j, 65] -> out [65, i] = attn^T rows 0..63 plus
    the softmax denominator in row 64, accumulated over j tiles in PSUM
  - normalize via 1/x = exp(-ln x) on ScalarE with a DRAM-bounce partition
    broadcast; attn^T written in bf16 (head B of each pair lands on
    partitions 64..127 via a small DMA shift)
  - output projection contracts attn^T pair-tiles against woT k-tiles

Measured on 8 NeuronCores: HW exec ~416 us, max rel err ~3.9e-3 vs the
fp32 JAX reference (bf16 input-rounding noise floor).
"""

import numpy as np
import ml_dtypes

import concourse.bass as bass
import concourse.tile as tile
import concourse.mybir as mybir

F32 = mybir.dt.float32
BF16 = mybir.dt.bfloat16
F32R = mybir.dt.float32r

B, T, D = 4, 2048, 1024
NUM_HEADS, HEAD_DIM = 16, 64
ROPE_THETA = 10000.0

G = 512          # head dims per core (8 heads)
HPC = 8          # heads per core
PAIRS = 4        # pair-tiles (2 heads / 128 partitions)
KT = D // 128    # k-tiles over D
TC = 512         # i-chunk width
NCHUNK = T // TC
JT = T // 128    # j-tiles
TT = T // 128    # t-tiles
N_CORES = 8

MASK_NEG = -1.0e30

# Projection / output matmul input dtype. bf16: full PE rate.
DT_PROJ = BF16
# Scores (q,k) matmul dtype. bf16 enables row-packing two K=64 heads into
# one PE pass via tile_position (f32r packing mis-executes on HW).
DT_SCORE = BF16
# probs & v dtype for the AV matmul.
DT_AV = BF16


def _split_multi_waits(nc, max_waits=1):
    """This walrus build rejects >1 sync-wait per instruction; spill extras
    onto same-engine NoOps placed just before."""
    counter = [0]
    for func in nc.m.functions:
        for bb in func.blocks:
            insts = bb.instructions
            if not any(
                ins.sync_info is not None and len(ins.sync_info.on_wait) > max_waits
                for ins in insts
            ):
                continue
            new_list = []
            for ins in insts:
                si = ins.sync_info
                if si is None or len(si.on_wait) <= max_waits:
                    new_list.append(ins)
                    continue
                waits = list(si.on_wait)
                spill, keep = waits[:-max_waits], waits[-max_waits:]
                for w in spill:
                    counter[0] += 1
                    new_list.append(
                        mybir.InstNoOp(
                            name=f"waitnop-{counter[0]}",
                            engine=ins.engine,
                            ins=[],
                            outs=[],
                            sync_info=mybir.SyncInfo(on_wait=[w], on_update=[]),
                        )
                    )
                ins.sync_info = mybir.SyncInfo(on_wait=keep, on_update=list(si.on_update))
                new_list.append(ins)
            bb.instructions = new_list


def build_kernel():
    nc = bass.Bass()

    xT = nc.dram_tensor("xT", [D, T], DT_PROJ, kind="ExternalInput")
    wqT = nc.dram_tensor("wqT", [D, G], DT_PROJ, kind="ExternalInput")
    wkT = nc.dram_tensor("wkT", [D, G], DT_PROJ, kind="ExternalInput")
    wvT = nc.dram_tensor("wvT", [D, G], DT_PROJ, kind="ExternalInput")
    woT = nc.dram_tensor("woT", [G, D], DT_PROJ, kind="ExternalInput")
    cos2 = nc.dram_tensor("cos2", [128, T], F32, kind="ExternalInput")
    sin2 = nc.dram_tensor("sin2", [128, T], F32, kind="ExternalInput")
    trimask = nc.dram_tensor("trimask", [128, 128], F32, kind="ExternalInput")
    out = nc.dram_tensor("out", [T, D], F32, kind="ExternalOutput")

    with tile.TileContext(nc) as tc:
        with (
            tc.tile_pool(name="const", bufs=1) as cpool,
            tc.tile_pool(name="qk", bufs=1) as qkpool,
            tc.tile_pool(name="vext", bufs=1) as vpool,
            tc.tile_pool(name="attn", bufs=1) as apool,
            tc.tile_pool(name="rope", bufs=2) as rpool,
            tc.tile_pool(name="exps", bufs=6) as epool,
            tc.tile_pool(name="norm", bufs=3) as npool,
            tc.tile_pool(name="outp", bufs=1) as opool,
            tc.tile_pool(name="dramb", bufs=2, space="DRAM") as dpool,
            tc.tile_pool(name="mm", bufs=2, space="PSUM") as mmps,
            tc.tile_pool(name="sp", bufs=2, space="PSUM") as spps,
            tc.tile_pool(name="ap", bufs=2, space="PSUM") as apps,
        ):
            # ---- resident loads ----
            wq_sb = cpool.tile([128, KT, G], DT_PROJ, name="wq_sb")
            nc.sync.dma_start(wq_sb[:], wqT.rearrange("(k p) g -> p k g", p=128))
            wk_sb = cpool.tile([128, KT, G], DT_PROJ, name="wk_sb")
            nc.sync.dma_start(wk_sb[:], wkT.rearrange("(k p) g -> p k g", p=128))
            wv_sb = cpool.tile([128, KT, G], DT_PROJ, name="wv_sb")
            nc.sync.dma_start(wv_sb[:], wvT.rearrange("(k p) g -> p k g", p=128))
            wo_sb = cpool.tile([128, PAIRS, D], DT_PROJ, name="wo_sb")
            nc.sync.dma_start(wo_sb[:], woT.rearrange("(k p) d -> p k d", p=128))
            cos_sb = cpool.tile([128, T], F32, name="cos_sb")
            sin_sb = cpool.tile([128, T], F32, name="sin_sb")
            nc.sync.dma_start(cos_sb[:], cos2[:])
            nc.sync.dma_start(sin_sb[:], sin2[:])
            tri_sb = cpool.tile([128, 128], F32, name="tri_sb")
            nc.sync.dma_start(tri_sb[:], trimask[:])
            sgn_sb = cpool.tile([128, 1], F32, name="sgn_sb")
            nc.vector.memset(sgn_sb[0:32, :], -1.0)
            nc.vector.memset(sgn_sb[32:64, :], 1.0)
            nc.vector.memset(sgn_sb[64:96, :], -1.0)
            nc.vector.memset(sgn_sb[96:128, :], 1.0)

            qrot = qkpool.tile([128, PAIRS, T], DT_SCORE, name="qrot")
            krot = qkpool.tile([128, PAIRS, T], DT_SCORE, name="krot")
            v_ext = vpool.tile([128, TT, HPC, 65], DT_AV, name="v_ext")
            attnT = apool.tile([128, PAIRS, T], DT_PROJ, name="attnT")

            # ---- projections + RoPE, streaming xT per T-chunk ----
            xT_r = xT.rearrange("(k p) t -> p k t", p=128)
            for c in range(NCHUNK):
                csl = bass.ts(c, TC)
                xc = rpool.tile([128, KT, TC], DT_PROJ, name="xc", tag="xc")
                nc.sync.dma_start(xc[:], xT_r[:, :, csl])
                for p in range(PAIRS):
                    for w_sb, rot in ((wq_sb, qrot), (wk_sb, krot)):
                        ps = mmps.tile([128, TC], F32, name="proj_ps", tag="mmps")
                        for k in range(KT):
                            nc.tensor.matmul(
                                ps[:],
                                w_sb[:, k, bass.ts(p, 128)],
                                xc[:, k, :],
                                start=(k == 0),
                                stop=(k == KT - 1),
                            )
                        # rope: rot = ps*cos + swap(ps)*sgn*sin
                        pf = rpool.tile([128, TC], F32, name="pf", tag="pf")
                        nc.vector.tensor_copy(pf[:], ps[:])
                        sw = rpool.tile([128, TC], F32, name="sw", tag="sw")
                        for blk in range(4):
                            src = (blk ^ 1) * 32
                            nc.sync.dma_start(
                                sw[blk * 32 : blk * 32 + 32, :],
                                pf[src : src + 32, :],
                            )
                        t2 = rpool.tile([128, TC], F32, name="t2", tag="t2")
                        nc.vector.tensor_mul(t2[:], ps[:], cos_sb[:, csl])
                        nc.vector.scalar_tensor_tensor(
                            out=sw[:],
                            in0=sw[:],
                            scalar=sgn_sb[:],
                            in1=sin_sb[:, csl],
                            op0=mybir.AluOpType.mult,
                            op1=mybir.AluOpType.mult,
                        )
                        nc.vector.tensor_add(rot[:, p, csl], sw[:], t2[:])
                # V projection for this chunk's 4 t-tiles (natural layout)
                for tt in range(4):
                    t = 4 * c + tt
                    ps = mmps.tile([128, G], F32, name="v_ps", tag="mmps")
                    for k in range(KT):
                        nc.tensor.matmul(
                            ps[:],
                            xc[:, k, bass.ts(tt, 128)],
                            wv_sb[:, k, :],
                            start=(k == 0),
                            stop=(k == KT - 1),
                        )
                    nc.vector.tensor_copy(
                        v_ext[:, t, :, 0:64],
                        ps[:].rearrange("p (h d) -> p h d", h=HPC),
                    )
                    nc.vector.memset(v_ext[:, t, :, 64:65], 1.0)

            # ---- attention per (pair, chunk), both heads row-packed ----
            # The two K=64 score matmuls of a head pair run concurrently in
            # the PE array via tile_position row groups (0,0)/(64,0).
            for p in range(PAIRS):
                for c in range(NCHUNK):
                    csl = bass.ts(c, TC)
                    atts = [
                        apps.tile([65, TC], F32, name=f"att{hh}_ps", tag="apps")
                        for hh in range(2)
                    ]
                    njt = 4 * c + 4
                    for jt in range(njt):
                        m = jt - 4 * c  # >=0 on diagonal j-tiles
                        # both heads' scores in one 2-bank psum tile so a
                        # single FD-doubled exp covers the pair
                        sAB = spps.tile([128, 2, TC], F32, name="s_ps", tag="spps")
                        soff = 128 * m if m > 0 else 0
                        for hh in range(2):
                            hsl = slice(64 * hh, 64 * hh + 64)
                            nc.tensor.matmul(
                                sAB[:, hh, soff:TC],
                                krot[hsl, p, bass.ts(jt, 128)],
                                qrot[hsl, p, c * TC + soff : (c + 1) * TC],
                                start=True,
                                stop=True,
                                tile_position=(64 * hh, 0),
                            )
                        off = 128 * m
                        fd = TC - off
                        if m >= 0:
                            for hh in range(2):
                                nc.vector.tensor_add(
                                    sAB[:, hh, off : off + 128],
                                    sAB[:, hh, off : off + 128],
                                    tri_sb[:],
                                )
                            es = epool.tile([128, 2, TC], DT_AV, name="es", tag="es")
                            nc.scalar.activation(
                                es[:, :, 0:fd],
                                sAB[:, :, off : off + fd],
                                mybir.ActivationFunctionType.Exp,
                                scale=0.125,
                            )
                            for hh in range(2):
                                nc.tensor.matmul(
                                    atts[hh][:, off : off + fd],
                                    v_ext[:, jt, 2 * p + hh, :],
                                    es[:, hh, 0:fd],
                                    start=(jt == 0),
                                    stop=(jt == njt - 1),
                                )
                        else:
                            es = epool.tile([128, 2, TC], DT_AV, name="es", tag="es")
                            nc.scalar.activation(
                                es[:],
                                sAB[:],
                                mybir.ActivationFunctionType.Exp,
                                scale=0.125,
                            )
                            for hh in range(2):
                                nc.tensor.matmul(
                                    atts[hh][:],
                                    v_ext[:, jt, 2 * p + hh, :],
                                    es[:, hh, :],
                                    start=(jt == 0),
                                    stop=(jt == njt - 1),
                                )
                    # normalize both heads: 1/denom = exp(-ln(denom)) on ACT
                    # (custom-DVE reciprocal unsupported; engines lane-locked).
                    # Ln on the psum denom row (lane 64), DRAM bounce
                    # broadcasts to 64 partitions, Exp(scale=-1) gives 1/x.
                    for hh in range(2):
                        att = atts[hh]
                        nrm = npool.tile([128, TC], F32, name="nrm", tag="nrm")
                        nc.scalar.activation(
                            nrm[64:65, :],
                            att[64:65, :],
                            mybir.ActivationFunctionType.Ln,
                        )
                        rbc = npool.tile([64, TC], F32, name="rbc", tag="rbc")
                        dscr = dpool.tile([1, TC], F32, name="dscr", tag="dscr")
                        nc.sync.dma_start(dscr[:], nrm[64:65, :])
                        dsrc = dscr[:]
                        nc.sync.dma_start(
                            rbc[:],
                            bass.AP(
                                tensor=dsrc.tensor,
                                offset=dsrc.offset,
                                ap=[[0, 64]] + dsrc.ap[1:],
                            ),
                        )
                        nc.scalar.activation(
                            rbc[:],
                            rbc[:],
                            mybir.ActivationFunctionType.Exp,
                            scale=-1.0,
                        )
                        if hh == 0:
                            nc.vector.tensor_mul(
                                attnT[0:64, p, csl], att[0:64, :], rbc[:]
                            )
                        else:
                            btmp = npool.tile([64, TC], DT_PROJ, name="btmp", tag="btmp")
                            nc.vector.tensor_mul(btmp[:], att[0:64, :], rbc[:])
                            nc.sync.dma_start(attnT[64:128, p, csl], btmp[:])

            # ---- output projection ----
            for t in range(TT):
                tsl = bass.ts(t, 128)
                ob = opool.tile([128, D], F32, name="ob", tag="ob")
                for dc in range(2):
                    dsl = bass.ts(dc, 512)
                    ps = mmps.tile([128, 512], F32, name="o_ps", tag="mmps")
                    for p in range(PAIRS):
                        nc.tensor.matmul(
                            ps[:],
                            attnT[:, p, tsl],
                            wo_sb[:, p, dsl],
                            start=(p == 0),
                            stop=(p == PAIRS - 1),
                        )
                    nc.vector.tensor_copy(ob[:, dsl], ps[:])
                nc.sync.dma_start(out[t * 128 : t * 128 + 128, :], ob[:])

    _split_multi_waits(nc)
    return nc


def _round_tf32(x):
    u = np.ascontiguousarray(x, dtype=np.float32).view(np.uint32)
    rb = (u >> 13) & 1
    u = (u + 0x0FFF + rb) & np.uint32(0xFFFFE000)
    return u.view(np.float32)


def _to_dt(x, dt):
    if dt == BF16:
        return np.ascontiguousarray(x).astype(ml_dtypes.bfloat16)
    if dt == F32R:
        return _round_tf32(np.ascontiguousarray(x, dtype=np.float32))
    return np.ascontiguousarray(x, dtype=np.float32)


def _rope_tables():
    inv_freq = 1.0 / ROPE_THETA ** (np.arange(0, HEAD_DIM, 2, dtype=np.float64) / HEAD_DIM)
    freqs = np.outer(np.arange(T, dtype=np.float64), inv_freq)  # [T, 32]
    cos_t = np.cos(freqs).T.astype(np.float32)  # [32, T]
    sin_t = np.sin(freqs).T.astype(np.float32)
    cos2 = np.concatenate([cos_t, cos_t, cos_t, cos_t], axis=0)  # [128, T]
    sin2 = np.concatenate([sin_t, sin_t, sin_t, sin_t], axis=0)
    return np.ascontiguousarray(cos2), np.ascontiguousarray(sin2)


def _trimask():
    j = np.arange(128)[:, None]
    c = np.arange(128)[None, :]
    return np.where(j <= c, 0.0, MASK_NEG).astype(np.float32)


_NC_CACHE = {}
LAST_RESULTS = None  # BassKernelResults of the most recent kernel() call


def kernel(x, wq, wk, wv, wo):
    global LAST_RESULTS
    from concourse.bass_utils import run_bass_kernel_spmd

    x = np.asarray(x, dtype=np.float32)
    wq = np.asarray(wq, dtype=np.float32)
    wk = np.asarray(wk, dtype=np.float32)
    wv = np.asarray(wv, dtype=np.float32)
    wo = np.asarray(wo, dtype=np.float32)

    cos2, sin2 = _rope_tables()
    tri = _trimask()

    in_maps = []
    for core in range(N_CORES):
        b, g = core // 2, core % 2
        gs = slice(G * g, G * g + G)
        in_maps.append(
            {
                "xT": _to_dt(x[b].T, DT_PROJ),
                "wqT": _to_dt(wq[gs].T, DT_PROJ),
                "wkT": _to_dt(wk[gs].T, DT_PROJ),
                "wvT": _to_dt(wv[gs].T, DT_PROJ),
                "woT": _to_dt(wo[:, gs].T, DT_PROJ),
                "cos2": cos2,
                "sin2": sin2,
                "trimask": tri,
            }
        )

    if "nc" not in _NC_CACHE:
        _NC_CACHE["nc"] = build_kernel()
    nc = _NC_CACHE["nc"]

    res = run_bass_kernel_spmd(nc, in_maps, core_ids=list(range(N_CORES)))
    LAST_RESULTS = res
    outs = [r["out"] for r in res.results]
    full = np.empty((B, T, D), dtype=np.float32)
    for b in range(B):
        full[b] = (
            outs[2 * b].astype(np.float64) + outs[2 * b + 1].astype(np.float64)
        ).astype(np.float32)
    return full


# revision 29
# speedup vs baseline: 1.2249x; 1.0172x over previous
"""Causal self-attention with RoPE on 8 TRN2 NeuronCores.

Problem: B=4, T=2048, D=1024, 16 heads x 64 dims, fp32, causal, RoPE.

Sharding: (batch b, head-group g) -> core b*2+g. Each core computes the
full sequence for 8 heads of one batch plus that group's partial output
projection; the host sums the two partial projections per batch
(the "all-reduce" of the tensor-parallel split, done host-side).

Per-core layout strategy (everything keeps the contraction dim on SBUF
partitions, so no on-device transposes are needed):
  - host supplies x^T [D, T] and pre-transposed weights wqT/wkT/wvT [D, 512],
    woT [512, D] in bf16
  - q^T, k^T computed as [512 hdim, T] (pair-tiles of 128 partitions =
    2 heads x 64 dims); v computed in natural [T, 512] layout directly
  - RoPE applied to q^T/k^T in fp32: partition-half swap via SBUF->SBUF DMA,
    then two multiplies + add on DVE with host cos/sin tables
  - scores S^T[j,i] = k^T.T @ q^T per head; the two K=64 heads of a pair run
    concurrently in the PE array via tile_position row groups (0,0)/(64,0)
  - exp on ScalarE (scale=1/8 folded in, bf16 out); causal diagonal blocks
    masked by adding a -1e30 triangle to the first 128 columns pre-exp
  - AV: lhsT = [v_h | ones] [j, 65] -> out [65, i] = attn^T rows 0..63 plus
    the softmax denominator in row 64, accumulated over j tiles in PSUM
  - normalize via 1/x = exp(-ln x) on ScalarE with a DRAM-bounce partition
    broadcast; attn^T written in bf16 (head B of each pair lands on
    partitions 64..127 via a small DMA shift)
  - output projection contracts attn^T pair-tiles against woT k-tiles

Measured on 8 NeuronCores: HW exec ~416 us, max rel err ~3.9e-3 vs the
fp32 JAX reference (bf16 input-rounding noise floor).
"""

import numpy as np
import ml_dtypes

import concourse.bass as bass
import concourse.tile as tile
import concourse.mybir as mybir

F32 = mybir.dt.float32
BF16 = mybir.dt.bfloat16
F32R = mybir.dt.float32r

B, T, D = 4, 2048, 1024
NUM_HEADS, HEAD_DIM = 16, 64
ROPE_THETA = 10000.0

G = 512          # head dims per core (8 heads)
HPC = 8          # heads per core
PAIRS = 4        # pair-tiles (2 heads / 128 partitions)
KT = D // 128    # k-tiles over D
TC = 512         # i-chunk width
NCHUNK = T // TC
JT = T // 128    # j-tiles
TT = T // 128    # t-tiles
N_CORES = 8

MASK_NEG = -1.0e30

# Projection / output matmul input dtype. bf16: full PE rate.
DT_PROJ = BF16
# Scores (q,k) matmul dtype. bf16 enables row-packing two K=64 heads into
# one PE pass via tile_position (f32r packing mis-executes on HW).
DT_SCORE = BF16
# probs & v dtype for the AV matmul.
DT_AV = BF16


def _split_multi_waits(nc, max_waits=1):
    """This walrus build rejects >1 sync-wait per instruction; spill extras
    onto same-engine NoOps placed just before."""
    counter = [0]
    for func in nc.m.functions:
        for bb in func.blocks:
            insts = bb.instructions
            if not any(
                ins.sync_info is not None and len(ins.sync_info.on_wait) > max_waits
                for ins in insts
            ):
                continue
            new_list = []
            for ins in insts:
                si = ins.sync_info
                if si is None or len(si.on_wait) <= max_waits:
                    new_list.append(ins)
                    continue
                waits = list(si.on_wait)
                spill, keep = waits[:-max_waits], waits[-max_waits:]
                for w in spill:
                    counter[0] += 1
                    new_list.append(
                        mybir.InstNoOp(
                            name=f"waitnop-{counter[0]}",
                            engine=ins.engine,
                            ins=[],
                            outs=[],
                            sync_info=mybir.SyncInfo(on_wait=[w], on_update=[]),
                        )
                    )
                ins.sync_info = mybir.SyncInfo(on_wait=keep, on_update=list(si.on_update))
                new_list.append(ins)
            bb.instructions = new_list


def build_kernel():
    nc = bass.Bass()

    xT = nc.dram_tensor("xT", [D, T], DT_PROJ, kind="ExternalInput")
    wqT = nc.dram_tensor("wqT", [D, G], DT_PROJ, kind="ExternalInput")
    wkT = nc.dram_tensor("wkT", [D, G], DT_PROJ, kind="ExternalInput")
    wvT = nc.dram_tensor("wvT", [D, G], DT_PROJ, kind="ExternalInput")
    woT = nc.dram_tensor("woT", [G, D], DT_PROJ, kind="ExternalInput")
    cos2 = nc.dram_tensor("cos2", [128, T], F32, kind="ExternalInput")
    sin2 = nc.dram_tensor("sin2", [128, T], F32, kind="ExternalInput")
    trimask = nc.dram_tensor("trimask", [128, 128], F32, kind="ExternalInput")
    out = nc.dram_tensor("out", [T, D], F32, kind="ExternalOutput")

    with tile.TileContext(nc) as tc:
        with (
            tc.tile_pool(name="const", bufs=1) as cpool,
            tc.tile_pool(name="qk", bufs=1) as qkpool,
            tc.tile_pool(name="vext", bufs=1) as vpool,
            tc.tile_pool(name="attn", bufs=1) as apool,
            tc.tile_pool(name="rope", bufs=2) as rpool,
            tc.tile_pool(name="exps", bufs=6) as epool,
            tc.tile_pool(name="norm", bufs=3) as npool,
            tc.tile_pool(name="outp", bufs=1) as opool,
            tc.tile_pool(name="dramb", bufs=2, space="DRAM") as dpool,
            tc.tile_pool(name="mm", bufs=2, space="PSUM") as mmps,
            tc.tile_pool(name="sp", bufs=2, space="PSUM") as spps,
            tc.tile_pool(name="ap", bufs=2, space="PSUM") as apps,
        ):
            # ---- resident loads ----
            wq_sb = cpool.tile([128, KT, G], DT_PROJ, name="wq_sb")
            nc.sync.dma_start(wq_sb[:], wqT.rearrange("(k p) g -> p k g", p=128))
            wk_sb = cpool.tile([128, KT, G], DT_PROJ, name="wk_sb")
            nc.sync.dma_start(wk_sb[:], wkT.rearrange("(k p) g -> p k g", p=128))
            wv_sb = cpool.tile([128, KT, G], DT_PROJ, name="wv_sb")
            nc.sync.dma_start(wv_sb[:], wvT.rearrange("(k p) g -> p k g", p=128))
            wo_sb = cpool.tile([128, PAIRS, D], DT_PROJ, name="wo_sb")
            nc.sync.dma_start(wo_sb[:], woT.rearrange("(k p) d -> p k d", p=128))
            cos_sb = cpool.tile([128, T], F32, name="cos_sb")
            sin_sb = cpool.tile([128, T], F32, name="sin_sb")
            nc.sync.dma_start(cos_sb[:], cos2[:])
            nc.sync.dma_start(sin_sb[:], sin2[:])
            tri_sb = cpool.tile([128, 128], F32, name="tri_sb")
            nc.sync.dma_start(tri_sb[:], trimask[:])
            sgn_sb = cpool.tile([128, 1], F32, name="sgn_sb")
            nc.vector.memset(sgn_sb[0:32, :], -1.0)
            nc.vector.memset(sgn_sb[32:64, :], 1.0)
            nc.vector.memset(sgn_sb[64:96, :], -1.0)
            nc.vector.memset(sgn_sb[96:128, :], 1.0)

            qrot = qkpool.tile([128, PAIRS, T], DT_SCORE, name="qrot")
            krot = qkpool.tile([128, PAIRS, T], DT_SCORE, name="krot")
            v_ext = vpool.tile([128, TT, HPC, 65], DT_AV, name="v_ext")
            attnT = apool.tile([128, PAIRS, T], DT_PROJ, name="attnT")

            # ---- projections + RoPE, streaming xT per T-chunk ----
            xT_r = xT.rearrange("(k p) t -> p k t", p=128)
            for c in range(NCHUNK):
                csl = bass.ts(c, TC)
                xc = rpool.tile([128, KT, TC], DT_PROJ, name="xc", tag="xc")
                nc.sync.dma_start(xc[:], xT_r[:, :, csl])
                for p in range(PAIRS):
                    for w_sb, rot in ((wq_sb, qrot), (wk_sb, krot)):
                        ps = mmps.tile([128, TC], F32, name="proj_ps", tag="mmps")
                        for k in range(KT):
                            nc.tensor.matmul(
                                ps[:],
                                w_sb[:, k, bass.ts(p, 128)],
                                xc[:, k, :],
                                start=(k == 0),
                                stop=(k == KT - 1),
                            )
                        # rope: rot = ps*cos + swap(ps)*sgn*sin
                        pf = rpool.tile([128, TC], F32, name="pf", tag="pf")
                        nc.vector.tensor_copy(pf[:], ps[:])
                        sw = rpool.tile([128, TC], F32, name="sw", tag="sw")
                        for blk in range(4):
                            src = (blk ^ 1) * 32
                            nc.sync.dma_start(
                                sw[blk * 32 : blk * 32 + 32, :],
                                pf[src : src + 32, :],
                            )
                        t2 = rpool.tile([128, TC], F32, name="t2", tag="t2")
                        nc.vector.tensor_mul(t2[:], ps[:], cos_sb[:, csl])
                        nc.vector.scalar_tensor_tensor(
                            out=sw[:],
                            in0=sw[:],
                            scalar=sgn_sb[:],
                            in1=sin_sb[:, csl],
                            op0=mybir.AluOpType.mult,
                            op1=mybir.AluOpType.mult,
                        )
                        nc.vector.tensor_add(rot[:, p, csl], sw[:], t2[:])
                # V projection for this chunk's 4 t-tiles (natural layout)
                for tt in range(4):
                    t = 4 * c + tt
                    ps = mmps.tile([128, G], F32, name="v_ps", tag="mmps")
                    for k in range(KT):
                        nc.tensor.matmul(
                            ps[:],
                            xc[:, k, bass.ts(tt, 128)],
                            wv_sb[:, k, :],
                            start=(k == 0),
                            stop=(k == KT - 1),
                        )
                    nc.vector.tensor_copy(
                        v_ext[:, t, :, 0:64],
                        ps[:].rearrange("p (h d) -> p h d", h=HPC),
                    )
                    nc.vector.memset(v_ext[:, t, :, 64:65], 1.0)

            # ---- attention per (pair, chunk), both heads row-packed ----
            # The two K=64 score matmuls of a head pair run concurrently in
            # the PE array via tile_position row groups (0,0)/(64,0).
            for p in range(PAIRS):
                for c in range(NCHUNK):
                    csl = bass.ts(c, TC)
                    atts = [
                        apps.tile([65, TC], F32, name=f"att{hh}_ps", tag="apps")
                        for hh in range(2)
                    ]
                    njt = 4 * c + 4
                    for jt in range(njt):
                        m = jt - 4 * c  # >=0 on diagonal j-tiles
                        # both heads' scores in one 2-bank psum tile so a
                        # single FD-doubled exp covers the pair
                        sAB = spps.tile([128, 2, TC], F32, name="s_ps", tag="spps")
                        soff = 128 * m if m > 0 else 0
                        for hh in range(2):
                            hsl = slice(64 * hh, 64 * hh + 64)
                            nc.tensor.matmul(
                                sAB[:, hh, soff:TC],
                                krot[hsl, p, bass.ts(jt, 128)],
                                qrot[hsl, p, c * TC + soff : (c + 1) * TC],
                                start=True,
                                stop=True,
                                tile_position=(64 * hh, 0),
                            )
                        off = 128 * m
                        fd = TC - off
                        if m >= 0:
                            for hh in range(2):
                                nc.vector.tensor_add(
                                    sAB[:, hh, off : off + 128],
                                    sAB[:, hh, off : off + 128],
                                    tri_sb[:],
                                )
                            es = epool.tile([128, 2, TC], DT_AV, name="es", tag="es")
                            nc.scalar.activation(
                                es[:, :, 0:fd],
                                sAB[:, :, off : off + fd],
                                mybir.ActivationFunctionType.Exp,
                                scale=0.125,
                            )
                            for hh in range(2):
                                nc.tensor.matmul(
                                    atts[hh][:, off : off + fd],
                                    v_ext[:, jt, 2 * p + hh, :],
                                    es[:, hh, 0:fd],
                                    start=(jt == 0),
                                    stop=(jt == njt - 1),
                                )
                        else:
                            es = epool.tile([128, 2, TC], DT_AV, name="es", tag="es")
                            nc.scalar.activation(
                                es[:],
                                sAB[:],
                                mybir.ActivationFunctionType.Exp,
                                scale=0.125,
                            )
                            for hh in range(2):
                                nc.tensor.matmul(
                                    atts[hh][:],
                                    v_ext[:, jt, 2 * p + hh, :],
                                    es[:, hh, :],
                                    start=(jt == 0),
                                    stop=(jt == njt - 1),
                                )
                    # normalize both heads: 1/denom = exp(-ln(denom)) on ACT
                    # (custom-DVE reciprocal unsupported; engines lane-locked).
                    # Ln on the psum denom row (lane 64), DRAM bounce
                    # broadcasts to 64 partitions, Exp(scale=-1) gives 1/x.
                    for hh in range(2):
                        att = atts[hh]
                        # evacuate psum immediately so the next chunk's AV can
                        # reuse the bank; normalize runs from SBUF off the
                        # critical path
                        attU = npool.tile([65, TC], F32, name="attU", tag="attU")
                        nc.vector.tensor_copy(attU[:], att[:])
                        att = attU
                        nrm = npool.tile([128, TC], F32, name="nrm", tag="nrm")
                        nc.scalar.activation(
                            nrm[64:65, :],
                            att[64:65, :],
                            mybir.ActivationFunctionType.Ln,
                        )
                        rbc = npool.tile([64, TC], F32, name="rbc", tag="rbc")
                        dscr = dpool.tile([1, TC], F32, name="dscr", tag="dscr")
                        nc.sync.dma_start(dscr[:], nrm[64:65, :])
                        dsrc = dscr[:]
                        nc.sync.dma_start(
                            rbc[:],
                            bass.AP(
                                tensor=dsrc.tensor,
                                offset=dsrc.offset,
                                ap=[[0, 64]] + dsrc.ap[1:],
                            ),
                        )
                        nc.scalar.activation(
                            rbc[:],
                            rbc[:],
                            mybir.ActivationFunctionType.Exp,
                            scale=-1.0,
                        )
                        if hh == 0:
                            nc.vector.tensor_mul(
                                attnT[0:64, p, csl], att[0:64, :], rbc[:]
                            )
                        else:
                            btmp = npool.tile([64, TC], DT_PROJ, name="btmp", tag="btmp")
                            nc.vector.tensor_mul(btmp[:], att[0:64, :], rbc[:])
                            nc.sync.dma_start(attnT[64:128, p, csl], btmp[:])

            # ---- output projection ----
            for t in range(TT):
                tsl = bass.ts(t, 128)
                ob = opool.tile([128, D], F32, name="ob", tag="ob")
                for dc in range(2):
                    dsl = bass.ts(dc, 512)
                    ps = mmps.tile([128, 512], F32, name="o_ps", tag="mmps")
                    for p in range(PAIRS):
                        nc.tensor.matmul(
                            ps[:],
                            attnT[:, p, tsl],
                            wo_sb[:, p, dsl],
                            start=(p == 0),
                            stop=(p == PAIRS - 1),
                        )
                    nc.vector.tensor_copy(ob[:, dsl], ps[:])
                nc.sync.dma_start(out[t * 128 : t * 128 + 128, :], ob[:])

    _split_multi_waits(nc)
    return nc


def _round_tf32(x):
    u = np.ascontiguousarray(x, dtype=np.float32).view(np.uint32)
    rb = (u >> 13) & 1
    u = (u + 0x0FFF + rb) & np.uint32(0xFFFFE000)
    return u.view(np.float32)


def _to_dt(x, dt):
    if dt == BF16:
        return np.ascontiguousarray(x).astype(ml_dtypes.bfloat16)
    if dt == F32R:
        return _round_tf32(np.ascontiguousarray(x, dtype=np.float32))
    return np.ascontiguousarray(x, dtype=np.float32)


def _rope_tables():
    inv_freq = 1.0 / ROPE_THETA ** (np.arange(0, HEAD_DIM, 2, dtype=np.float64) / HEAD_DIM)
    freqs = np.outer(np.arange(T, dtype=np.float64), inv_freq)  # [T, 32]
    cos_t = np.cos(freqs).T.astype(np.float32)  # [32, T]
    sin_t = np.sin(freqs).T.astype(np.float32)
    cos2 = np.concatenate([cos_t, cos_t, cos_t, cos_t], axis=0)  # [128, T]
    sin2 = np.concatenate([sin_t, sin_t, sin_t, sin_t], axis=0)
    return np.ascontiguousarray(cos2), np.ascontiguousarray(sin2)


def _trimask():
    j = np.arange(128)[:, None]
    c = np.arange(128)[None, :]
    return np.where(j <= c, 0.0, MASK_NEG).astype(np.float32)


_NC_CACHE = {}
LAST_RESULTS = None  # BassKernelResults of the most recent kernel() call


def kernel(x, wq, wk, wv, wo):
    global LAST_RESULTS
    from concourse.bass_utils import run_bass_kernel_spmd

    x = np.asarray(x, dtype=np.float32)
    wq = np.asarray(wq, dtype=np.float32)
    wk = np.asarray(wk, dtype=np.float32)
    wv = np.asarray(wv, dtype=np.float32)
    wo = np.asarray(wo, dtype=np.float32)

    cos2, sin2 = _rope_tables()
    tri = _trimask()

    in_maps = []
    for core in range(N_CORES):
        b, g = core // 2, core % 2
        gs = slice(G * g, G * g + G)
        in_maps.append(
            {
                "xT": _to_dt(x[b].T, DT_PROJ),
                "wqT": _to_dt(wq[gs].T, DT_PROJ),
                "wkT": _to_dt(wk[gs].T, DT_PROJ),
                "wvT": _to_dt(wv[gs].T, DT_PROJ),
                "woT": _to_dt(wo[:, gs].T, DT_PROJ),
                "cos2": cos2,
                "sin2": sin2,
                "trimask": tri,
            }
        )

    if "nc" not in _NC_CACHE:
        _NC_CACHE["nc"] = build_kernel()
    nc = _NC_CACHE["nc"]

    res = run_bass_kernel_spmd(nc, in_maps, core_ids=list(range(N_CORES)))
    LAST_RESULTS = res
    outs = [r["out"] for r in res.results]
    full = np.empty((B, T, D), dtype=np.float32)
    for b in range(B):
        full[b] = (
            outs[2 * b].astype(np.float64) + outs[2 * b + 1].astype(np.float64)
        ).astype(np.float32)
    return full


# revision 31
# speedup vs baseline: 1.2643x; 1.0321x over previous
"""Causal self-attention with RoPE on 8 TRN2 NeuronCores.

Problem: B=4, T=2048, D=1024, 16 heads x 64 dims, fp32, causal, RoPE.

Sharding: (batch b, head-group g) -> core b*2+g. Each core computes the
full sequence for 8 heads of one batch plus that group's partial output
projection; the host sums the two partial projections per batch
(the "all-reduce" of the tensor-parallel split, done host-side).

Per-core layout strategy (everything keeps the contraction dim on SBUF
partitions, so no on-device transposes are needed):
  - host supplies x^T [D, T] and pre-transposed weights wqT/wkT/wvT [D, 512],
    woT [512, D] in bf16
  - q^T, k^T computed as [512 hdim, T] (pair-tiles of 128 partitions =
    2 heads x 64 dims); v computed in natural [T, 512] layout directly
  - RoPE applied to q^T/k^T in fp32: partition-half swap via SBUF->SBUF DMA,
    then two multiplies + add on DVE with host cos/sin tables
  - scores S^T[j,i] = k^T.T @ q^T per head; the two K=64 heads of a pair run
    concurrently in the PE array via tile_position row groups (0,0)/(64,0)
  - exp on ScalarE (scale=1/8 folded in, bf16 out); causal diagonal blocks
    masked by adding a -1e30 triangle to the first 128 columns pre-exp
  - AV: lhsT = [v_h | ones] [j, 65] -> out [65, i] = attn^T rows 0..63 plus
    the softmax denominator in row 64, accumulated over j tiles in PSUM
  - normalize via 1/x = exp(-ln x) on ScalarE with a DRAM-bounce partition
    broadcast; attn^T written in bf16 (head B of each pair lands on
    partitions 64..127 via a small DMA shift)
  - output projection contracts attn^T pair-tiles against woT k-tiles

Measured on 8 NeuronCores: HW exec ~407 us, max rel err ~3.9e-3 vs the
fp32 JAX reference (bf16 input-rounding noise floor).
"""

import numpy as np
import ml_dtypes

import concourse.bass as bass
import concourse.tile as tile
import concourse.mybir as mybir

F32 = mybir.dt.float32
BF16 = mybir.dt.bfloat16
F32R = mybir.dt.float32r

B, T, D = 4, 2048, 1024
NUM_HEADS, HEAD_DIM = 16, 64
ROPE_THETA = 10000.0

G = 512          # head dims per core (8 heads)
HPC = 8          # heads per core
PAIRS = 4        # pair-tiles (2 heads / 128 partitions)
KT = D // 128    # k-tiles over D
TC = 512         # i-chunk width
NCHUNK = T // TC
JT = T // 128    # j-tiles
TT = T // 128    # t-tiles
N_CORES = 8

MASK_NEG = -1.0e30

# Projection / output matmul input dtype. bf16: full PE rate.
DT_PROJ = BF16
# Scores (q,k) matmul dtype. bf16 enables row-packing two K=64 heads into
# one PE pass via tile_position (f32r packing mis-executes on HW).
DT_SCORE = BF16
# probs & v dtype for the AV matmul.
DT_AV = BF16


def _split_multi_waits(nc, max_waits=1):
    """This walrus build rejects >1 sync-wait per instruction; spill extras
    onto same-engine NoOps placed just before."""
    counter = [0]
    for func in nc.m.functions:
        for bb in func.blocks:
            insts = bb.instructions
            if not any(
                ins.sync_info is not None and len(ins.sync_info.on_wait) > max_waits
                for ins in insts
            ):
                continue
            new_list = []
            for ins in insts:
                si = ins.sync_info
                if si is None or len(si.on_wait) <= max_waits:
                    new_list.append(ins)
                    continue
                waits = list(si.on_wait)
                spill, keep = waits[:-max_waits], waits[-max_waits:]
                for w in spill:
                    counter[0] += 1
                    new_list.append(
                        mybir.InstNoOp(
                            name=f"waitnop-{counter[0]}",
                            engine=ins.engine,
                            ins=[],
                            outs=[],
                            sync_info=mybir.SyncInfo(on_wait=[w], on_update=[]),
                        )
                    )
                ins.sync_info = mybir.SyncInfo(on_wait=keep, on_update=list(si.on_update))
                new_list.append(ins)
            bb.instructions = new_list


def build_kernel():
    nc = bass.Bass()

    xT = nc.dram_tensor("xT", [D, T], DT_PROJ, kind="ExternalInput")
    wqT = nc.dram_tensor("wqT", [D, G], DT_PROJ, kind="ExternalInput")
    wkT = nc.dram_tensor("wkT", [D, G], DT_PROJ, kind="ExternalInput")
    wvT = nc.dram_tensor("wvT", [D, G], DT_PROJ, kind="ExternalInput")
    woT = nc.dram_tensor("woT", [G, D], DT_PROJ, kind="ExternalInput")
    cos2 = nc.dram_tensor("cos2", [128, T], F32, kind="ExternalInput")
    sin2 = nc.dram_tensor("sin2", [128, T], F32, kind="ExternalInput")
    trimask = nc.dram_tensor("trimask", [128, 128], F32, kind="ExternalInput")
    out = nc.dram_tensor("out", [T, D], F32, kind="ExternalOutput")

    with tile.TileContext(nc) as tc:
        with (
            tc.tile_pool(name="const", bufs=1) as cpool,
            tc.tile_pool(name="qk", bufs=1) as qkpool,
            tc.tile_pool(name="vext", bufs=1) as vpool,
            tc.tile_pool(name="attn", bufs=1) as apool,
            tc.tile_pool(name="rope", bufs=2) as rpool,
            tc.tile_pool(name="exps", bufs=6) as epool,
            tc.tile_pool(name="norm", bufs=4) as npool,
            tc.tile_pool(name="outp", bufs=2) as opool,
            tc.tile_pool(name="dramb", bufs=2, space="DRAM") as dpool,
            tc.tile_pool(name="mm", bufs=2, space="PSUM") as mmps,
            tc.tile_pool(name="sp", bufs=2, space="PSUM") as spps,
            tc.tile_pool(name="ap", bufs=2, space="PSUM") as apps,
        ):
            # ---- resident loads ----
            wq_sb = cpool.tile([128, KT, G], DT_PROJ, name="wq_sb")
            nc.sync.dma_start(wq_sb[:], wqT.rearrange("(k p) g -> p k g", p=128))
            wk_sb = cpool.tile([128, KT, G], DT_PROJ, name="wk_sb")
            nc.sync.dma_start(wk_sb[:], wkT.rearrange("(k p) g -> p k g", p=128))
            wv_sb = cpool.tile([128, KT, G], DT_PROJ, name="wv_sb")
            nc.sync.dma_start(wv_sb[:], wvT.rearrange("(k p) g -> p k g", p=128))
            wo_sb = cpool.tile([128, PAIRS, D], DT_PROJ, name="wo_sb")
            nc.sync.dma_start(wo_sb[:], woT.rearrange("(k p) d -> p k d", p=128))
            cos_sb = cpool.tile([128, T], F32, name="cos_sb")
            sin_sb = cpool.tile([128, T], F32, name="sin_sb")
            nc.sync.dma_start(cos_sb[:], cos2[:])
            nc.sync.dma_start(sin_sb[:], sin2[:])
            tri_sb = cpool.tile([128, 128], F32, name="tri_sb")
            nc.sync.dma_start(tri_sb[:], trimask[:])
            sgn_sb = cpool.tile([128, 1], F32, name="sgn_sb")
            nc.vector.memset(sgn_sb[0:32, :], -1.0)
            nc.vector.memset(sgn_sb[32:64, :], 1.0)
            nc.vector.memset(sgn_sb[64:96, :], -1.0)
            nc.vector.memset(sgn_sb[96:128, :], 1.0)

            qrot = qkpool.tile([128, PAIRS, T], DT_SCORE, name="qrot")
            krot = qkpool.tile([128, PAIRS, T], DT_SCORE, name="krot")
            v_ext = vpool.tile([128, TT, HPC, 65], DT_AV, name="v_ext")
            attnT = apool.tile([128, PAIRS, T], DT_PROJ, name="attnT")

            # ---- projections + RoPE, streaming xT per T-chunk ----
            xT_r = xT.rearrange("(k p) t -> p k t", p=128)
            for c in range(NCHUNK):
                csl = bass.ts(c, TC)
                xc = rpool.tile([128, KT, TC], DT_PROJ, name="xc", tag="xc")
                nc.sync.dma_start(xc[:], xT_r[:, :, csl])
                for p in range(PAIRS):
                    for w_sb, rot in ((wq_sb, qrot), (wk_sb, krot)):
                        ps = mmps.tile([128, TC], F32, name="proj_ps", tag="mmps")
                        for k in range(KT):
                            nc.tensor.matmul(
                                ps[:],
                                w_sb[:, k, bass.ts(p, 128)],
                                xc[:, k, :],
                                start=(k == 0),
                                stop=(k == KT - 1),
                            )
                        # rope: rot = ps*cos + swap(ps)*sgn*sin
                        pf = rpool.tile([128, TC], F32, name="pf", tag="pf")
                        nc.vector.tensor_copy(pf[:], ps[:])
                        sw = rpool.tile([128, TC], F32, name="sw", tag="sw")
                        for blk in range(4):
                            src = (blk ^ 1) * 32
                            nc.sync.dma_start(
                                sw[blk * 32 : blk * 32 + 32, :],
                                pf[src : src + 32, :],
                            )
                        t2 = rpool.tile([128, TC], F32, name="t2", tag="t2")
                        nc.vector.tensor_mul(t2[:], ps[:], cos_sb[:, csl])
                        nc.vector.scalar_tensor_tensor(
                            out=sw[:],
                            in0=sw[:],
                            scalar=sgn_sb[:],
                            in1=sin_sb[:, csl],
                            op0=mybir.AluOpType.mult,
                            op1=mybir.AluOpType.mult,
                        )
                        nc.vector.tensor_add(rot[:, p, csl], sw[:], t2[:])
                # V projection for this chunk's 4 t-tiles (natural layout)
                for tt in range(4):
                    t = 4 * c + tt
                    ps = mmps.tile([128, G], F32, name="v_ps", tag="mmps")
                    for k in range(KT):
                        nc.tensor.matmul(
                            ps[:],
                            xc[:, k, bass.ts(tt, 128)],
                            wv_sb[:, k, :],
                            start=(k == 0),
                            stop=(k == KT - 1),
                        )
                    nc.vector.tensor_copy(
                        v_ext[:, t, :, 0:64],
                        ps[:].rearrange("p (h d) -> p h d", h=HPC),
                    )
                    nc.vector.memset(v_ext[:, t, :, 64:65], 1.0)

            # ---- attention per (pair, chunk), both heads row-packed ----
            # The two K=64 score matmuls of a head pair run concurrently in
            # the PE array via tile_position row groups (0,0)/(64,0).
            for p in range(PAIRS):
                for c in range(NCHUNK):
                    csl = bass.ts(c, TC)
                    atts = [
                        apps.tile([65, TC], F32, name=f"att{hh}_ps", tag="apps")
                        for hh in range(2)
                    ]
                    njt = 4 * c + 4
                    for jt in range(njt):
                        m = jt - 4 * c  # >=0 on diagonal j-tiles
                        # both heads' scores in one 2-bank psum tile so a
                        # single FD-doubled exp covers the pair
                        sAB = spps.tile([128, 2, TC], F32, name="s_ps", tag="spps")
                        soff = 128 * m if m > 0 else 0
                        for hh in range(2):
                            hsl = slice(64 * hh, 64 * hh + 64)
                            nc.tensor.matmul(
                                sAB[:, hh, soff:TC],
                                krot[hsl, p, bass.ts(jt, 128)],
                                qrot[hsl, p, c * TC + soff : (c + 1) * TC],
                                start=True,
                                stop=True,
                                tile_position=(64 * hh, 0),
                            )
                        off = 128 * m
                        fd = TC - off
                        if m >= 0:
                            for hh in range(2):
                                nc.vector.tensor_add(
                                    sAB[:, hh, off : off + 128],
                                    sAB[:, hh, off : off + 128],
                                    tri_sb[:],
                                )
                            es = epool.tile([128, 2, TC], DT_AV, name="es", tag="es")
                            nc.scalar.activation(
                                es[:, :, 0:fd],
                                sAB[:, :, off : off + fd],
                                mybir.ActivationFunctionType.Exp,
                                scale=0.125,
                            )
                            for hh in range(2):
                                nc.tensor.matmul(
                                    atts[hh][:, off : off + fd],
                                    v_ext[:, jt, 2 * p + hh, :],
                                    es[:, hh, 0:fd],
                                    start=(jt == 0),
                                    stop=(jt == njt - 1),
                                )
                        else:
                            es = epool.tile([128, 2, TC], DT_AV, name="es", tag="es")
                            nc.scalar.activation(
                                es[:],
                                sAB[:],
                                mybir.ActivationFunctionType.Exp,
                                scale=0.125,
                            )
                            for hh in range(2):
                                nc.tensor.matmul(
                                    atts[hh][:],
                                    v_ext[:, jt, 2 * p + hh, :],
                                    es[:, hh, :],
                                    start=(jt == 0),
                                    stop=(jt == njt - 1),
                                )
                    # normalize both heads: 1/denom = exp(-ln(denom)) on ACT
                    # (custom-DVE reciprocal unsupported; engines lane-locked).
                    # Ln on the psum denom row (lane 64), DRAM bounce
                    # broadcasts to 64 partitions, Exp(scale=-1) gives 1/x.
                    for hh in range(2):
                        att = atts[hh]
                        # evacuate psum immediately so the next chunk's AV can
                        # reuse the bank; normalize runs from SBUF off the
                        # critical path
                        attU = npool.tile([65, TC], F32, name="attU", tag="attU")
                        nc.vector.tensor_copy(attU[:], att[:])
                        att = attU
                        nrm = npool.tile([128, TC], F32, name="nrm", tag="nrm")
                        nc.scalar.activation(
                            nrm[64:65, :],
                            att[64:65, :],
                            mybir.ActivationFunctionType.Ln,
                        )
                        rbc = npool.tile([64, TC], F32, name="rbc", tag="rbc")
                        dscr = dpool.tile([1, TC], F32, name="dscr", tag="dscr")
                        nc.sync.dma_start(dscr[:], nrm[64:65, :])
                        dsrc = dscr[:]
                        nc.sync.dma_start(
                            rbc[:],
                            bass.AP(
                                tensor=dsrc.tensor,
                                offset=dsrc.offset,
                                ap=[[0, 64]] + dsrc.ap[1:],
                            ),
                        )
                        nc.scalar.activation(
                            rbc[:],
                            rbc[:],
                            mybir.ActivationFunctionType.Exp,
                            scale=-1.0,
                        )
                        if hh == 0:
                            nc.vector.tensor_mul(
                                attnT[0:64, p, csl], att[0:64, :], rbc[:]
                            )
                        else:
                            btmp = npool.tile([64, TC], DT_PROJ, name="btmp", tag="btmp")
                            nc.vector.tensor_mul(btmp[:], att[0:64, :], rbc[:])
                            nc.sync.dma_start(attnT[64:128, p, csl], btmp[:])

            # ---- output projection ----
            for t in range(TT):
                tsl = bass.ts(t, 128)
                ob = opool.tile([128, D], F32, name="ob", tag="ob")
                for dc in range(2):
                    dsl = bass.ts(dc, 512)
                    ps = mmps.tile([128, 512], F32, name="o_ps", tag="mmps")
                    for p in range(PAIRS):
                        nc.tensor.matmul(
                            ps[:],
                            attnT[:, p, tsl],
                            wo_sb[:, p, dsl],
                            start=(p == 0),
                            stop=(p == PAIRS - 1),
                        )
                    nc.vector.tensor_copy(ob[:, dsl], ps[:])
                nc.sync.dma_start(out[t * 128 : t * 128 + 128, :], ob[:])

    _split_multi_waits(nc)
    return nc


def _round_tf32(x):
    u = np.ascontiguousarray(x, dtype=np.float32).view(np.uint32)
    rb = (u >> 13) & 1
    u = (u + 0x0FFF + rb) & np.uint32(0xFFFFE000)
    return u.view(np.float32)


def _to_dt(x, dt):
    if dt == BF16:
        return np.ascontiguousarray(x).astype(ml_dtypes.bfloat16)
    if dt == F32R:
        return _round_tf32(np.ascontiguousarray(x, dtype=np.float32))
    return np.ascontiguousarray(x, dtype=np.float32)


def _rope_tables():
    inv_freq = 1.0 / ROPE_THETA ** (np.arange(0, HEAD_DIM, 2, dtype=np.float64) / HEAD_DIM)
    freqs = np.outer(np.arange(T, dtype=np.float64), inv_freq)  # [T, 32]
    cos_t = np.cos(freqs).T.astype(np.float32)  # [32, T]
    sin_t = np.sin(freqs).T.astype(np.float32)
    cos2 = np.concatenate([cos_t, cos_t, cos_t, cos_t], axis=0)  # [128, T]
    sin2 = np.concatenate([sin_t, sin_t, sin_t, sin_t], axis=0)
    return np.ascontiguousarray(cos2), np.ascontiguousarray(sin2)


def _trimask():
    j = np.arange(128)[:, None]
    c = np.arange(128)[None, :]
    return np.where(j <= c, 0.0, MASK_NEG).astype(np.float32)


_NC_CACHE = {}
LAST_RESULTS = None  # BassKernelResults of the most recent kernel() call


def kernel(x, wq, wk, wv, wo):
    global LAST_RESULTS
    from concourse.bass_utils import run_bass_kernel_spmd

    x = np.asarray(x, dtype=np.float32)
    wq = np.asarray(wq, dtype=np.float32)
    wk = np.asarray(wk, dtype=np.float32)
    wv = np.asarray(wv, dtype=np.float32)
    wo = np.asarray(wo, dtype=np.float32)

    cos2, sin2 = _rope_tables()
    tri = _trimask()

    in_maps = []
    for core in range(N_CORES):
        b, g = core // 2, core % 2
        gs = slice(G * g, G * g + G)
        in_maps.append(
            {
                "xT": _to_dt(x[b].T, DT_PROJ),
                "wqT": _to_dt(wq[gs].T, DT_PROJ),
                "wkT": _to_dt(wk[gs].T, DT_PROJ),
                "wvT": _to_dt(wv[gs].T, DT_PROJ),
                "woT": _to_dt(wo[:, gs].T, DT_PROJ),
                "cos2": cos2,
                "sin2": sin2,
                "trimask": tri,
            }
        )

    if "nc" not in _NC_CACHE:
        _NC_CACHE["nc"] = build_kernel()
    nc = _NC_CACHE["nc"]

    res = run_bass_kernel_spmd(nc, in_maps, core_ids=list(range(N_CORES)))
    LAST_RESULTS = res
    outs = [r["out"] for r in res.results]
    full = np.empty((B, T, D), dtype=np.float32)
    for b in range(B):
        full[b] = (
            outs[2 * b].astype(np.float64) + outs[2 * b + 1].astype(np.float64)
        ).astype(np.float32)
    return full


# revision 33
# speedup vs baseline: 1.3228x; 1.0463x over previous
"""Causal self-attention with RoPE on 8 TRN2 NeuronCores.

Problem: B=4, T=2048, D=1024, 16 heads x 64 dims, fp32, causal, RoPE.

Sharding: (batch b, head-group g) -> core b*2+g. Each core computes the
full sequence for 8 heads of one batch plus that group's partial output
projection; the host sums the two partial projections per batch
(the "all-reduce" of the tensor-parallel split, done host-side).

Per-core layout strategy (everything keeps the contraction dim on SBUF
partitions, so no on-device transposes are needed):
  - host supplies x^T [D, T] and pre-transposed weights wqT/wkT/wvT [D, 512],
    woT [512, D] in bf16
  - q^T, k^T computed as [512 hdim, T] (pair-tiles of 128 partitions =
    2 heads x 64 dims); v computed in natural [T, 512] layout directly
  - RoPE applied to q^T/k^T in fp32: partition-half swap via SBUF->SBUF DMA,
    then two multiplies + add on DVE with host cos/sin tables
  - scores S^T[j,i] = k^T.T @ q^T per head; the two K=64 heads of a pair run
    concurrently in the PE array via tile_position row groups (0,0)/(64,0)
  - exp on ScalarE (scale=1/8 folded in, bf16 out); causal diagonal blocks
    masked by adding a -1e30 triangle to the first 128 columns pre-exp
  - AV: lhsT = [v_h | ones] [j, 65] -> out [65, i] = attn^T rows 0..63 plus
    the softmax denominator in row 64, accumulated over j tiles in PSUM
  - normalize via 1/x = exp(-ln x) on ScalarE with a DRAM-bounce partition
    broadcast; attn^T written in bf16 (head B of each pair lands on
    partitions 64..127 via a small DMA shift)
  - output projection contracts attn^T pair-tiles against woT k-tiles

Measured on 8 NeuronCores: HW exec ~394 us, max rel err ~3.9e-3 vs the
fp32 JAX reference (bf16 input-rounding noise floor).
"""

import numpy as np
import ml_dtypes

import concourse.bass as bass
import concourse.tile as tile
import concourse.mybir as mybir

F32 = mybir.dt.float32
BF16 = mybir.dt.bfloat16
F32R = mybir.dt.float32r

B, T, D = 4, 2048, 1024
NUM_HEADS, HEAD_DIM = 16, 64
ROPE_THETA = 10000.0

G = 512          # head dims per core (8 heads)
HPC = 8          # heads per core
PAIRS = 4        # pair-tiles (2 heads / 128 partitions)
KT = D // 128    # k-tiles over D
TC = 512         # i-chunk width
NCHUNK = T // TC
JT = T // 128    # j-tiles
TT = T // 128    # t-tiles
N_CORES = 8

MASK_NEG = -1.0e30

# Projection / output matmul input dtype. bf16: full PE rate.
DT_PROJ = BF16
# Scores (q,k) matmul dtype. bf16 enables row-packing two K=64 heads into
# one PE pass via tile_position (f32r packing mis-executes on HW).
DT_SCORE = BF16
# probs & v dtype for the AV matmul.
DT_AV = BF16


def _split_multi_waits(nc, max_waits=1):
    """This walrus build rejects >1 sync-wait per instruction; spill extras
    onto same-engine NoOps placed just before."""
    counter = [0]
    for func in nc.m.functions:
        for bb in func.blocks:
            insts = bb.instructions
            if not any(
                ins.sync_info is not None and len(ins.sync_info.on_wait) > max_waits
                for ins in insts
            ):
                continue
            new_list = []
            for ins in insts:
                si = ins.sync_info
                if si is None or len(si.on_wait) <= max_waits:
                    new_list.append(ins)
                    continue
                waits = list(si.on_wait)
                spill, keep = waits[:-max_waits], waits[-max_waits:]
                for w in spill:
                    counter[0] += 1
                    new_list.append(
                        mybir.InstNoOp(
                            name=f"waitnop-{counter[0]}",
                            engine=ins.engine,
                            ins=[],
                            outs=[],
                            sync_info=mybir.SyncInfo(on_wait=[w], on_update=[]),
                        )
                    )
                ins.sync_info = mybir.SyncInfo(on_wait=keep, on_update=list(si.on_update))
                new_list.append(ins)
            bb.instructions = new_list


def build_kernel():
    nc = bass.Bass()

    xT = nc.dram_tensor("xT", [D, T], DT_PROJ, kind="ExternalInput")
    wqT = nc.dram_tensor("wqT", [D, G], DT_PROJ, kind="ExternalInput")
    wkT = nc.dram_tensor("wkT", [D, G], DT_PROJ, kind="ExternalInput")
    wvT = nc.dram_tensor("wvT", [D, G], DT_PROJ, kind="ExternalInput")
    woT = nc.dram_tensor("woT", [G, D], DT_PROJ, kind="ExternalInput")
    cos2 = nc.dram_tensor("cos2", [128, T], F32, kind="ExternalInput")
    sin2 = nc.dram_tensor("sin2", [128, T], F32, kind="ExternalInput")
    trimask = nc.dram_tensor("trimask", [128, 128], F32, kind="ExternalInput")
    out = nc.dram_tensor("out", [T, D], F32, kind="ExternalOutput")

    with tile.TileContext(nc) as tc:
        with (
            tc.tile_pool(name="const", bufs=1) as cpool,
            tc.tile_pool(name="qk", bufs=1) as qkpool,
            tc.tile_pool(name="vext", bufs=1) as vpool,
            tc.tile_pool(name="attn", bufs=1) as apool,
            tc.tile_pool(name="rope", bufs=2) as rpool,
            tc.tile_pool(name="exps", bufs=8) as epool,
            tc.tile_pool(name="norm", bufs=4) as npool,
            tc.tile_pool(name="outp", bufs=2) as opool,
            tc.tile_pool(name="dramb", bufs=2, space="DRAM") as dpool,
            tc.tile_pool(name="mm", bufs=2, space="PSUM") as mmps,
            tc.tile_pool(name="sp", bufs=2, space="PSUM") as spps,
            tc.tile_pool(name="ap", bufs=2, space="PSUM") as apps,
        ):
            # ---- resident loads ----
            wq_sb = cpool.tile([128, KT, G], DT_PROJ, name="wq_sb")
            nc.sync.dma_start(wq_sb[:], wqT.rearrange("(k p) g -> p k g", p=128))
            wk_sb = cpool.tile([128, KT, G], DT_PROJ, name="wk_sb")
            nc.sync.dma_start(wk_sb[:], wkT.rearrange("(k p) g -> p k g", p=128))
            wv_sb = cpool.tile([128, KT, G], DT_PROJ, name="wv_sb")
            nc.sync.dma_start(wv_sb[:], wvT.rearrange("(k p) g -> p k g", p=128))
            wo_sb = cpool.tile([128, PAIRS, D], DT_PROJ, name="wo_sb")
            nc.sync.dma_start(wo_sb[:], woT.rearrange("(k p) d -> p k d", p=128))
            cos_sb = cpool.tile([128, T], F32, name="cos_sb")
            sin_sb = cpool.tile([128, T], F32, name="sin_sb")
            nc.sync.dma_start(cos_sb[:], cos2[:])
            nc.sync.dma_start(sin_sb[:], sin2[:])
            tri_sb = cpool.tile([128, 128], F32, name="tri_sb")
            nc.sync.dma_start(tri_sb[:], trimask[:])
            sgn_sb = cpool.tile([128, 1], F32, name="sgn_sb")
            nc.vector.memset(sgn_sb[0:32, :], -1.0)
            nc.vector.memset(sgn_sb[32:64, :], 1.0)
            nc.vector.memset(sgn_sb[64:96, :], -1.0)
            nc.vector.memset(sgn_sb[96:128, :], 1.0)

            qrot = qkpool.tile([128, PAIRS, T], DT_SCORE, name="qrot")
            krot = qkpool.tile([128, PAIRS, T], DT_SCORE, name="krot")
            v_ext = vpool.tile([128, TT, HPC, 65], DT_AV, name="v_ext")
            attnT = apool.tile([128, PAIRS, T], DT_PROJ, name="attnT")

            # ---- projections + RoPE, streaming xT per T-chunk ----
            xT_r = xT.rearrange("(k p) t -> p k t", p=128)
            for c in range(NCHUNK):
                csl = bass.ts(c, TC)
                xc = rpool.tile([128, KT, TC], DT_PROJ, name="xc", tag="xc")
                nc.sync.dma_start(xc[:], xT_r[:, :, csl])
                for p in range(PAIRS):
                    for w_sb, rot in ((wq_sb, qrot), (wk_sb, krot)):
                        ps = mmps.tile([128, TC], F32, name="proj_ps", tag="mmps")
                        for k in range(KT):
                            nc.tensor.matmul(
                                ps[:],
                                w_sb[:, k, bass.ts(p, 128)],
                                xc[:, k, :],
                                start=(k == 0),
                                stop=(k == KT - 1),
                            )
                        # rope: rot = ps*cos + swap(ps)*sgn*sin
                        pf = rpool.tile([128, TC], F32, name="pf", tag="pf")
                        nc.vector.tensor_copy(pf[:], ps[:])
                        sw = rpool.tile([128, TC], F32, name="sw", tag="sw")
                        for blk in range(4):
                            src = (blk ^ 1) * 32
                            nc.sync.dma_start(
                                sw[blk * 32 : blk * 32 + 32, :],
                                pf[src : src + 32, :],
                            )
                        t2 = rpool.tile([128, TC], F32, name="t2", tag="t2")
                        nc.vector.tensor_mul(t2[:], ps[:], cos_sb[:, csl])
                        nc.vector.scalar_tensor_tensor(
                            out=sw[:],
                            in0=sw[:],
                            scalar=sgn_sb[:],
                            in1=sin_sb[:, csl],
                            op0=mybir.AluOpType.mult,
                            op1=mybir.AluOpType.mult,
                        )
                        nc.vector.tensor_add(rot[:, p, csl], sw[:], t2[:])
                # V projection for this chunk's 4 t-tiles (natural layout)
                for tt in range(4):
                    t = 4 * c + tt
                    ps = mmps.tile([128, G], F32, name="v_ps", tag="mmps")
                    for k in range(KT):
                        nc.tensor.matmul(
                            ps[:],
                            xc[:, k, bass.ts(tt, 128)],
                            wv_sb[:, k, :],
                            start=(k == 0),
                            stop=(k == KT - 1),
                        )
                    nc.vector.tensor_copy(
                        v_ext[:, t, :, 0:64],
                        ps[:].rearrange("p (h d) -> p h d", h=HPC),
                    )
                    nc.vector.memset(v_ext[:, t, :, 64:65], 1.0)

            # ---- attention per (chunk, pair), both heads row-packed ----
            # The two K=64 score matmuls of a head pair run concurrently in
            # the PE array via tile_position row groups (0,0)/(64,0).
            # Chunk-outer order interleaves the four pairs' independent work
            # so one pair's normalize tail overlaps another pair's scores.
            for c in range(NCHUNK):
                for p in range(PAIRS):
                    csl = bass.ts(c, TC)
                    atts = [
                        apps.tile([65, TC], F32, name=f"att{hh}_ps", tag="apps")
                        for hh in range(2)
                    ]
                    njt = 4 * c + 4
                    for jt in range(njt):
                        m = jt - 4 * c  # >=0 on diagonal j-tiles
                        # both heads' scores in one 2-bank psum tile so a
                        # single FD-doubled exp covers the pair
                        sAB = spps.tile([128, 2, TC], F32, name="s_ps", tag="spps")
                        soff = 128 * m if m > 0 else 0
                        for hh in range(2):
                            hsl = slice(64 * hh, 64 * hh + 64)
                            nc.tensor.matmul(
                                sAB[:, hh, soff:TC],
                                krot[hsl, p, bass.ts(jt, 128)],
                                qrot[hsl, p, c * TC + soff : (c + 1) * TC],
                                start=True,
                                stop=True,
                                tile_position=(64 * hh, 0),
                            )
                        off = 128 * m
                        fd = TC - off
                        if m >= 0:
                            for hh in range(2):
                                nc.vector.tensor_add(
                                    sAB[:, hh, off : off + 128],
                                    sAB[:, hh, off : off + 128],
                                    tri_sb[:],
                                )
                            es = epool.tile([128, 2, TC], DT_AV, name="es", tag="es")
                            nc.scalar.activation(
                                es[:, :, 0:fd],
                                sAB[:, :, off : off + fd],
                                mybir.ActivationFunctionType.Exp,
                                scale=0.125,
                            )
                            for hh in range(2):
                                nc.tensor.matmul(
                                    atts[hh][:, off : off + fd],
                                    v_ext[:, jt, 2 * p + hh, :],
                                    es[:, hh, 0:fd],
                                    start=(jt == 0),
                                    stop=(jt == njt - 1),
                                )
                        else:
                            es = epool.tile([128, 2, TC], DT_AV, name="es", tag="es")
                            nc.scalar.activation(
                                es[:],
                                sAB[:],
                                mybir.ActivationFunctionType.Exp,
                                scale=0.125,
                            )
                            for hh in range(2):
                                nc.tensor.matmul(
                                    atts[hh][:],
                                    v_ext[:, jt, 2 * p + hh, :],
                                    es[:, hh, :],
                                    start=(jt == 0),
                                    stop=(jt == njt - 1),
                                )
                    # normalize both heads: 1/denom = exp(-ln(denom)) on ACT
                    # (custom-DVE reciprocal unsupported; engines lane-locked).
                    # Ln on the psum denom row (lane 64), DRAM bounce
                    # broadcasts to 64 partitions, Exp(scale=-1) gives 1/x.
                    for hh in range(2):
                        att = atts[hh]
                        # evacuate psum immediately so the next chunk's AV can
                        # reuse the bank; normalize runs from SBUF off the
                        # critical path
                        attU = npool.tile([65, TC], F32, name="attU", tag="attU")
                        nc.vector.tensor_copy(attU[:], att[:])
                        att = attU
                        nrm = npool.tile([128, TC], F32, name="nrm", tag="nrm")
                        nc.scalar.activation(
                            nrm[64:65, :],
                            att[64:65, :],
                            mybir.ActivationFunctionType.Ln,
                        )
                        rbc = npool.tile([64, TC], F32, name="rbc", tag="rbc")
                        dscr = dpool.tile([1, TC], F32, name="dscr", tag="dscr")
                        nc.sync.dma_start(dscr[:], nrm[64:65, :])
                        dsrc = dscr[:]
                        nc.sync.dma_start(
                            rbc[:],
                            bass.AP(
                                tensor=dsrc.tensor,
                                offset=dsrc.offset,
                                ap=[[0, 64]] + dsrc.ap[1:],
                            ),
                        )
                        nc.scalar.activation(
                            rbc[:],
                            rbc[:],
                            mybir.ActivationFunctionType.Exp,
                            scale=-1.0,
                        )
                        if hh == 0:
                            nc.vector.tensor_mul(
                                attnT[0:64, p, csl], att[0:64, :], rbc[:]
                            )
                        else:
                            btmp = npool.tile([64, TC], DT_PROJ, name="btmp", tag="btmp")
                            nc.vector.tensor_mul(btmp[:], att[0:64, :], rbc[:])
                            nc.sync.dma_start(attnT[64:128, p, csl], btmp[:])

            # ---- output projection ----
            for t in range(TT):
                tsl = bass.ts(t, 128)
                ob = opool.tile([128, D], F32, name="ob", tag="ob")
                for dc in range(2):
                    dsl = bass.ts(dc, 512)
                    ps = mmps.tile([128, 512], F32, name="o_ps", tag="mmps")
                    for p in range(PAIRS):
                        nc.tensor.matmul(
                            ps[:],
                            attnT[:, p, tsl],
                            wo_sb[:, p, dsl],
                            start=(p == 0),
                            stop=(p == PAIRS - 1),
                        )
                    nc.vector.tensor_copy(ob[:, dsl], ps[:])
                nc.sync.dma_start(out[t * 128 : t * 128 + 128, :], ob[:])

    _split_multi_waits(nc)
    return nc


def _round_tf32(x):
    u = np.ascontiguousarray(x, dtype=np.float32).view(np.uint32)
    rb = (u >> 13) & 1
    u = (u + 0x0FFF + rb) & np.uint32(0xFFFFE000)
    return u.view(np.float32)


def _to_dt(x, dt):
    if dt == BF16:
        return np.ascontiguousarray(x).astype(ml_dtypes.bfloat16)
    if dt == F32R:
        return _round_tf32(np.ascontiguousarray(x, dtype=np.float32))
    return np.ascontiguousarray(x, dtype=np.float32)


def _rope_tables():
    inv_freq = 1.0 / ROPE_THETA ** (np.arange(0, HEAD_DIM, 2, dtype=np.float64) / HEAD_DIM)
    freqs = np.outer(np.arange(T, dtype=np.float64), inv_freq)  # [T, 32]
    cos_t = np.cos(freqs).T.astype(np.float32)  # [32, T]
    sin_t = np.sin(freqs).T.astype(np.float32)
    cos2 = np.concatenate([cos_t, cos_t, cos_t, cos_t], axis=0)  # [128, T]
    sin2 = np.concatenate([sin_t, sin_t, sin_t, sin_t], axis=0)
    return np.ascontiguousarray(cos2), np.ascontiguousarray(sin2)


def _trimask():
    j = np.arange(128)[:, None]
    c = np.arange(128)[None, :]
    return np.where(j <= c, 0.0, MASK_NEG).astype(np.float32)


_NC_CACHE = {}
LAST_RESULTS = None  # BassKernelResults of the most recent kernel() call


def kernel(x, wq, wk, wv, wo):
    global LAST_RESULTS
    from concourse.bass_utils import run_bass_kernel_spmd

    x = np.asarray(x, dtype=np.float32)
    wq = np.asarray(wq, dtype=np.float32)
    wk = np.asarray(wk, dtype=np.float32)
    wv = np.asarray(wv, dtype=np.float32)
    wo = np.asarray(wo, dtype=np.float32)

    cos2, sin2 = _rope_tables()
    tri = _trimask()

    in_maps = []
    for core in range(N_CORES):
        b, g = core // 2, core % 2
        gs = slice(G * g, G * g + G)
        in_maps.append(
            {
                "xT": _to_dt(x[b].T, DT_PROJ),
                "wqT": _to_dt(wq[gs].T, DT_PROJ),
                "wkT": _to_dt(wk[gs].T, DT_PROJ),
                "wvT": _to_dt(wv[gs].T, DT_PROJ),
                "woT": _to_dt(wo[:, gs].T, DT_PROJ),
                "cos2": cos2,
                "sin2": sin2,
                "trimask": tri,
            }
        )

    if "nc" not in _NC_CACHE:
        _NC_CACHE["nc"] = build_kernel()
    nc = _NC_CACHE["nc"]

    res = run_bass_kernel_spmd(nc, in_maps, core_ids=list(range(N_CORES)))
    LAST_RESULTS = res
    outs = [r["out"] for r in res.results]
    full = np.empty((B, T, D), dtype=np.float32)
    for b in range(B):
        full[b] = (
            outs[2 * b].astype(np.float64) + outs[2 * b + 1].astype(np.float64)
        ).astype(np.float32)
    return full
